# revision 1
# baseline (speedup 1.0000x reference)
"""Multi-head attention (degenerate multiplicative-mask softmax) on 8 TRN2 cores.

Sharding: pure data-parallel over batch (B=8 -> 1 batch element per core).
No collectives. Each core computes its batch's full attention + output proj.

Precision strategy (validated vs the f32 reference in numpy):
  - Q/K projections and Q@K^T: 3-pass bf16 hi/lo split (error ~2^-18 => zero
    argmax flips => exact one-hot softmax match).
  - V projection, P@V, output projection: single-pass bf16 (linear error
    ~0.3%, no argmax sensitivity).
  - Softmax degenerates: logits are (scores/8) * (mask * -1e9) ~ 1e9 scale,
    so exp(z - zmax) is an exact one-hot (top-2 logit gaps >> 88 for random
    scores); row sums are exactly 1.0 and the normalization divide is a no-op.
  - The logit product is computed as (qk * mask) * 1.25e8 which rounds
    identically to the reference's ((qk/8) * (mask * -1e9)) up to sign
    (1.25e8 and 1e9 share one mantissa), with the sign folded into a row-MIN
    reduction and exp(-u + umin).
"""
import sys

sys.path.insert(0, "/opt/trn_rl_repo")

import numpy as np
import ml_dtypes

import concourse.bass as bass
import concourse.tile as tile
from concourse import bacc, mybir
from concourse.bass_utils import run_bass_kernel_spmd

F32 = mybir.dt.float32
BF16 = mybir.dt.bfloat16
MULT = mybir.AluOpType.mult
MIN = mybir.AluOpType.min

B, S, D = 8, 1024, 1024
H, DH = 16, 64
P = 128
NT = S // P
SCALE = 1.25e8  # 1e9 / 8

_CACHE = {}


def _bf16(a):
    return np.ascontiguousarray(a.astype(ml_dtypes.bfloat16))


def _build(stage="full", net=16, nst=8, attn_mode="full", use_ttr=False):
    nc = bacc.Bacc(None)

    xh_d = nc.dram_tensor("xh", [D, S], BF16, kind="ExternalInput")  # x[b].T hi
    xl_d = nc.dram_tensor("xl", [D, S], BF16, kind="ExternalInput")  # x[b].T lo
    m_d = nc.dram_tensor("m", [S, S], BF16, kind="ExternalInput")  # mask [i, j] {0,1}
    wqkh_d = nc.dram_tensor("wqkh", [D, 2 * D], BF16, kind="ExternalInput")
    wqkl_d = nc.dram_tensor("wqkl", [D, 2 * D], BF16, kind="ExternalInput")
    wv_d = nc.dram_tensor("wv", [D, D], BF16, kind="ExternalInput")
    wp_d = nc.dram_tensor("wp", [D, D], BF16, kind="ExternalInput")
    bqkh_d = nc.dram_tensor("bqkh", [1, 2 * D], BF16, kind="ExternalInput")
    bqkl_d = nc.dram_tensor("bqkl", [1, 2 * D], BF16, kind="ExternalInput")
    bvh_d = nc.dram_tensor("bvh", [1, D], BF16, kind="ExternalInput")
    bvl_d = nc.dram_tensor("bvl", [1, D], BF16, kind="ExternalInput")
    bph_d = nc.dram_tensor("bph", [1, D], BF16, kind="ExternalInput")
    bpl_d = nc.dram_tensor("bpl", [1, D], BF16, kind="ExternalInput")
    id_d = nc.dram_tensor("ident", [P, P], BF16, kind="ExternalInput")
    y_d = nc.dram_tensor("y", [S, D], F32, kind="ExternalOutput")

    with tile.TileContext(nc) as tc:
        with (
            tc.tile_pool(name="res", bufs=1) as res,
            tc.tile_pool(name="qkres", bufs=1) as qkres,
            tc.tile_pool(name="vres", bufs=1) as vres,
            tc.tile_pool(name="psA", bufs=2, space="PSUM") as psA,
        ):
            # cross-phase residents
            mposb = res.tile([P, NT, S], BF16, tag="mposb")  # [i_sub, i_tile, j]
            nc.sync.dma_start(mposb[:], m_d.ap().rearrange("(t p) j -> p t j", p=P))
            otm = res.tile([P, NT, S], BF16, tag="otm")  # OT: [o_sub, o_tile, s]
            ones_row = res.tile([1, S], BF16, tag="ones")
            nc.vector.memset(ones_row[:], 1.0)
            ident = res.tile([P, P], BF16, tag="ident")
            nc.sync.dma_start(ident[:], id_d[:])
            biases = {}
            for nm, dd in (("bqkh", bqkh_d), ("bqkl", bqkl_d), ("bvh", bvh_d),
                           ("bvl", bvl_d), ("bph", bph_d), ("bpl", bpl_d)):
                t = res.tile([1, dd.shape[1]], BF16, tag=nm)
                nc.sync.dma_start(t[:], dd[:])
                biases[nm] = t

            # q/k hi+lo, resident through attention: [e_sub, e_tile, s]
            # e_tile 0..7 = q head-pairs, 8..15 = k head-pairs
            qkh = qkres.tile([P, 16, S], BF16, tag="qkh")
            qkl = qkres.tile([P, 16, S], BF16, tag="qkl")
            vmat = vres.tile([P, NT, D], BF16, tag="vmat")  # [j_sub, j_tile, c]

            # ---------------- phase 1+2: projections ----------------
            with tc.tile_pool(name="p12", bufs=1) as p12, \
                 tc.tile_pool(name="wstr", bufs=3) as wstr:
                xh = p12.tile([P, NT, S], BF16, tag="xh")  # [d_sub, d_tile, s]
                xl = p12.tile([P, NT, S], BF16, tag="xl")
                nc.sync.dma_start(xh[:], xh_d.ap().rearrange("(t p) s -> p t s", p=P))
                nc.sync.dma_start(xl[:], xl_d.ap().rearrange("(t p) s -> p t s", p=P))
                wv = p12.tile([P, NT, D], BF16, tag="wv")
                nc.sync.dma_start(wv[:], wv_d.ap().rearrange("(t p) c -> p t c", p=P))

                for et in range(net):
                    wh = wstr.tile([P, NT, P], BF16, tag="wh")
                    wl = wstr.tile([P, NT, P], BF16, tag="wl")
                    esl = slice(et * P, (et + 1) * P)
                    nc.sync.dma_start(
                        wh[:], wqkh_d[:, esl].rearrange("(t p) e -> p t e", p=P))
                    nc.sync.dma_start(
                        wl[:], wqkl_d[:, esl].rearrange("(t p) e -> p t e", p=P))
                    for nh in range(2):
                        hsl = slice(nh * 512, (nh + 1) * 512)
                        ps = psA.tile([P, 512], F32, tag="ps")
                        first = True
                        for k in range(NT):
                            for (wt, xt) in ((wh, xh), (wl, xh), (wh, xl)):
                                nc.tensor.matmul(
                                    ps[:], wt[:, k, :], xt[:, k, hsl],
                                    start=first, stop=False)
                                first = False
                        nc.tensor.matmul(
                            ps[:], biases["bqkh"][:, esl], ones_row[:, hsl],
                            start=False, stop=False)
                        nc.tensor.matmul(
                            ps[:], biases["bqkl"][:, esl], ones_row[:, hsl],
                            start=False, stop=True)
                        nc.scalar.copy(qkh[:, et, hsl], ps[:])
                        nc.vector.tensor_sub(qkl[:, et, hsl], ps[:], qkh[:, et, hsl])

                for st in range(nst):
                    ssl = slice(st * P, (st + 1) * P)
                    for nh in range(2):
                        hsl = slice(nh * 512, (nh + 1) * 512)
                        ps = psA.tile([P, 512], F32, tag="ps")
                        first = True
                        for k in range(NT):
                            nc.tensor.matmul(
                                ps[:], xh[:, k, ssl], wv[:, k, hsl],
                                start=first, stop=False)
                            first = False
                        nc.tensor.matmul(
                            ps[:], ones_row[:, ssl], biases["bvh"][:, hsl],
                            start=False, stop=False)
                        nc.tensor.matmul(
                            ps[:], ones_row[:, ssl], biases["bvl"][:, hsl],
                            start=False, stop=True)
                        nc.scalar.copy(vmat[:, st, hsl], ps[:])

            if stage == "p12":
                with tc.tile_pool(name="dbg", bufs=2) as dbg:
                    for st in range(NT):
                        yt = dbg.tile([P, D], F32, tag="yt")
                        nc.scalar.copy(yt[:], vmat[:, st, :])
                        nc.sync.dma_start(y_d[st * P : (st + 1) * P, :], yt[:])

            # ---------------- phase 3: attention ----------------
            if stage in ("attn", "full"):
              with tc.tile_pool(name="stg", bufs=2) as stg, \
                 tc.tile_pool(name="ppool", bufs=3) as ppool, \
                 tc.tile_pool(name="ptpool", bufs=2) as ptpool, \
                 tc.tile_pool(name="ps_s", bufs=2, space="PSUM") as ps_s, \
                 tc.tile_pool(name="ps_tr", bufs=2, space="PSUM") as ps_tr, \
                 tc.tile_pool(name="ps_o", bufs=2, space="PSUM") as ps_o:
                for hp in range(8):
                    qt, kt = hp, 8 + hp
                    qhB = stg.tile([64, S], BF16, tag="qhB")
                    qlB = stg.tile([64, S], BF16, tag="qlB")
                    khB = stg.tile([64, S], BF16, tag="khB")
                    klB = stg.tile([64, S], BF16, tag="klB")
                    nc.sync.dma_start(qhB[:], qkh[64:128, qt, :])
                    nc.sync.dma_start(qlB[:], qkl[64:128, qt, :])
                    nc.sync.dma_start(khB[:], qkh[64:128, kt, :])
                    nc.sync.dma_start(klB[:], qkl[64:128, kt, :])
                    ptbs = []
                    for hh in range(2):
                        lo, hi = hh * 64, hh * 64 + 64
                        pass
                        ptb = ptpool.tile([P, NT, S], BF16, tag="ptb")
                        ptbs.append(ptb)
                        for it in range(NT):
                            isl = slice(it * P, (it + 1) * P)
                            ut = ppool.tile([P, S], F32, tag="ut")
                            umin0 = ppool.tile([P, 1], F32, tag="umin0")
                            umin = ppool.tile([P, 1], F32, tag="umin")
                            biast = ppool.tile([P, 1], F32, tag="biast")
                            for nh in range(2):
                                hsl = slice(nh * 512, (nh + 1) * 512)
                                pss = ps_s.tile([P, 512], F32, tag="pss")
                                first = True
                                if hh == 0:
                                    mm_ops = (
                                        (qkh[0:64, qt, isl], qkh[0:64, kt, hsl]),
                                        (qkl[0:64, qt, isl], qkh[0:64, kt, hsl]),
                                        (qkh[0:64, qt, isl], qkl[0:64, kt, hsl]))
                                else:
                                    mm_ops = (
                                        (qhB[:, isl], khB[:, hsl]),
                                        (qlB[:, isl], khB[:, hsl]),
                                        (qhB[:, isl], klB[:, hsl]))
                                for mi, (qq, kk) in enumerate(mm_ops):
                                    nc.tensor.matmul(
                                        pss[:], qq, kk,
                                        start=first, stop=(mi == 2))
                                    first = False
                                if use_ttr:
                                    nc.vector.tensor_tensor_reduce(
                                        out=ut[:, hsl], in0=pss[:],
                                        in1=mposb[:, it, hsl],
                                        scale=SCALE,
                                        scalar=(3.0e38 if nh == 0 else umin0[:]),
                                        op0=MULT, op1=MIN,
                                        accum_out=(umin0[:] if nh == 0 else umin[:]))
                                else:
                                    nc.vector.scalar_tensor_tensor(
                                        out=ut[:, hsl], in0=pss[:],
                                        scalar=SCALE,
                                        in1=mposb[:, it, hsl],
                                        op0=MULT, op1=MULT)
                            pt = ppool.tile([P, S], BF16, tag="pt")
                            if use_ttr:
                                nc.scalar.activation(
                                    out=pt[:], in_=ut[:],
                                    func=mybir.ActivationFunctionType.Exp,
                                    bias=umin[:], scale=-1.0)
                            else:
                                nc.vector.tensor_reduce(
                                    out=umin[:], in_=ut[:],
                                    axis=mybir.AxisListType.X, op=MIN)
                                nc.scalar.activation(
                                    out=pt[:], in_=ut[:],
                                    func=mybir.ActivationFunctionType.Exp,
                                    bias=umin[:], scale=-1.0)
                            for trh in range(2):
                                pstr = ps_tr.tile([P, 512], BF16, tag="pstr")
                                for jj in range(4):
                                    jt = trh * 4 + jj
                                    nc.tensor.transpose(
                                        pstr[:, jj * P : (jj + 1) * P],
                                        pt[:, jt * P : (jt + 1) * P],
                                        ident[:])
                                dst = ptb[:, trh * 4 : trh * 4 + 4, isl]
                                if (it + trh) % 2 == 0:
                                    nc.vector.tensor_copy(dst, pstr[:].rearrange(
                                        "p (j i) -> p j i", j=4))
                                else:
                                    nc.scalar.copy(dst, pstr[:].rearrange(
                                        "p (j i) -> p j i", j=4))
                    for hh in range(2):
                        lo, hi = hh * 64, hh * 64 + 64
                        csl = slice(hp * P + lo, hp * P + hi)
                        for nh in range(2):
                            hsl = slice(nh * 512, (nh + 1) * 512)
                            pso = ps_o.tile([64, 512], F32, tag="pso")
                            for jt in range(NT):
                                nc.tensor.matmul(
                                    pso[:],
                                    vmat[:, jt, csl],
                                    ptbs[hh][:, jt, hsl],
                                    start=(jt == 0), stop=(jt == NT - 1))
                            nc.scalar.copy(otm[lo:hi, hp, hsl], pso[:])

            if stage == "attn":
                with tc.tile_pool(name="dbg", bufs=2) as dbg:
                    for ot in range(NT):
                        yt = dbg.tile([P, D], F32, tag="yt")
                        nc.scalar.copy(yt[:], otm[:, ot, :])
                        nc.sync.dma_start(y_d[ot * P : (ot + 1) * P, :], yt[:])

            # ---------------- phase 4: output projection ----------------
            if stage == "full":
              with tc.tile_pool(name="proj", bufs=1) as proj, \
                 tc.tile_pool(name="ypool", bufs=2) as ypool:
                wpt = proj.tile([P, NT, D], BF16, tag="wp")
                nc.sync.dma_start(wpt[:], wp_d.ap().rearrange("(t p) d -> p t d", p=P))
                for st in range(NT):
                    ssl = slice(st * P, (st + 1) * P)
                    yt = ypool.tile([P, D], F32, tag="yt")
                    for nh in range(2):
                        hsl = slice(nh * 512, (nh + 1) * 512)
                        ps = psA.tile([P, 512], F32, tag="ps")
                        first = True
                        for ot in range(NT):
                            nc.tensor.matmul(
                                ps[:], otm[:, ot, ssl], wpt[:, ot, hsl],
                                start=first, stop=False)
                            first = False
                        nc.tensor.matmul(
                            ps[:], ones_row[:, ssl], biases["bph"][:, hsl],
                            start=False, stop=False)
                        nc.tensor.matmul(
                            ps[:], ones_row[:, ssl], biases["bpl"][:, hsl],
                            start=False, stop=True)
                        nc.scalar.copy(yt[:, hsl], ps[:])
                    nc.sync.dma_start(y_d[st * P : (st + 1) * P, :], yt[:])

    nc.compile()
    return nc


def _prep_inputs(x, mask, W_attn, b_attn, W_proj, b_proj):
    x = np.asarray(x, np.float32)
    mask = np.asarray(mask, np.float32)
    W_attn = np.asarray(W_attn, np.float32)
    b_attn = np.asarray(b_attn, np.float32).reshape(-1)
    W_proj = np.asarray(W_proj, np.float32)
    b_proj = np.asarray(b_proj, np.float32).reshape(-1)

    wqk = W_attn[:, : 2 * D]
    wqkh = _bf16(wqk)
    wqkl = _bf16(wqk - wqkh.astype(np.float32))
    wv = _bf16(W_attn[:, 2 * D :])
    wp = _bf16(W_proj)

    def split_row(v):
        r = v.reshape(1, -1)
        h = _bf16(r)
        l = _bf16(r - h.astype(np.float32))
        return h, l

    bqkh, bqkl = split_row(b_attn[: 2 * D])
    bvh, bvl = split_row(b_attn[2 * D :])
    bph, bpl = split_row(b_proj)

    shared = dict(wqkh=wqkh, wqkl=wqkl, wv=wv, wp=wp, bqkh=bqkh, bqkl=bqkl,
                  bvh=bvh, bvl=bvl, bph=bph, bpl=bpl,
                  ident=_bf16(np.eye(P, dtype=np.float32)))
    in_maps = []
    for b in range(B):
        xT = np.ascontiguousarray(x[b].T)
        xh = _bf16(xT)
        xli = _bf16(xT - xh.astype(np.float32))
        in_maps.append(dict(xh=xh, xl=xli, m=_bf16(mask[b, 0]), **shared))
    return in_maps


def kernel(x, mask, W_attn, b_attn, W_proj, b_proj, _trace=False, _trace_kwargs=None):
    if "nc" not in _CACHE:
        _CACHE["nc"] = _build()
    nc = _CACHE["nc"]
    in_maps = _prep_inputs(x, mask, W_attn, b_attn, W_proj, b_proj)
    kw = {}
    if _trace:
        kw = dict(trace=True, **(_trace_kwargs or {}))
    res = run_bass_kernel_spmd(nc, in_maps, core_ids=list(range(B)), **kw)
    out = np.stack([res.results[b]["y"] for b in range(B)], axis=0)
    if _trace:
        _CACHE["last_results"] = res
    return out



# revision 16
# speedup vs baseline: 1.1498x; 1.1498x over previous
"""Multi-head attention (degenerate multiplicative-mask softmax) on 8 TRN2 cores.

Sharding: pure data-parallel over batch (B=8 -> 1 batch element per core).
No collectives. Each core computes its batch's full attention + output proj.

v3 design (v1 757us -> v2 623us -> v3):
  - Scores: 3-pass bf16 hi/lo packed into 2 matmuls: K=128 [qh;ql]x[kh;kh]
    + K=64 qh x kl. Same numerics as v1, 2/3 the PE time.
  - P^T built by DMA xbar transposes (dma_start_transpose, 3D out) on the
    otherwise-idle DMA engines - no PE transposes, no psum->sbuf copies.
  - FULL proj/attention interleave: after each (q-et, k-et) projection pair,
    the two ready heads' attention is emitted, so vector/scalar/DMA attention
    work overlaps the projection matmul stream and TensorE becomes the only
    wall. SBUF fits because: masked scores live in PSUM (no ut tile), the
    attention output otm round-trips through a DRAM tile (saves 16K/part),
    mask is fp8, and phase-4 weights load into a late-scoped pool.
  - PV runs one head behind the scores pipeline (ptb double-buffered), so
    the in-order PE never stalls on the xbar transposes.
  - Biases folded: q/k exact-f32 via activation(Identity, bias) on the
    psum->sbuf copy; v/proj via single K=1 ones-matmul.

Precision: identical hi/lo 3-pass scheme as v1 (rel err 0.0054, all from the
single-bf16 V path; argmax flips ~0).
"""
import sys

sys.path.insert(0, "/opt/trn_rl_repo")

import numpy as np
import ml_dtypes

import concourse.bass as bass  # noqa: F401
import concourse.tile as tile
from concourse import bacc, mybir
from concourse.bass_utils import run_bass_kernel_spmd

F32 = mybir.dt.float32
FP8 = mybir.dt.float8e4
BF16 = mybir.dt.bfloat16
ADD = mybir.AluOpType.add
SUB = mybir.AluOpType.subtract
MULT = mybir.AluOpType.mult
MIN = mybir.AluOpType.min
IDENT = mybir.ActivationFunctionType.Identity
EXP = mybir.ActivationFunctionType.Exp
AX = mybir.AxisListType.X

B, S, D = 8, 1024, 1024
H, DH = 16, 64
P = 128
NT = S // P
SCALE = 1.25e8  # 1e9 / 8

_CACHE = {}


def _bf16(a):
    return np.ascontiguousarray(a.astype(ml_dtypes.bfloat16))


def _build():
    nc = bacc.Bacc(None)

    xh_d = nc.dram_tensor("xh", [D, S], BF16, kind="ExternalInput")  # x[b].T hi
    xl_d = nc.dram_tensor("xl", [D, S], BF16, kind="ExternalInput")  # x[b].T lo
    m_d = nc.dram_tensor("m", [S, S], FP8, kind="ExternalInput")  # mask [i, j]
    wqkh_d = nc.dram_tensor("wqkh", [D, 2 * D], BF16, kind="ExternalInput")
    wqkl_d = nc.dram_tensor("wqkl", [D, 2 * D], BF16, kind="ExternalInput")
    wv_d = nc.dram_tensor("wv", [D, D], BF16, kind="ExternalInput")
    wp_d = nc.dram_tensor("wp", [D, D], BF16, kind="ExternalInput")
    bqk_lo_d = nc.dram_tensor("bqk_lo", [64, 16], F32, kind="ExternalInput")
    bqk_hi_d = nc.dram_tensor("bqk_hi", [64, 16], F32, kind="ExternalInput")
    bv_d = nc.dram_tensor("bv", [1, D], BF16, kind="ExternalInput")
    bp_d = nc.dram_tensor("bp", [1, D], BF16, kind="ExternalInput")
    y_d = nc.dram_tensor("y", [S, D], F32, kind="ExternalOutput")

    with tile.TileContext(nc) as tc:
        with (
            tc.tile_pool(name="res", bufs=1) as res,
            tc.tile_pool(name="qkres", bufs=1) as qkres,
            tc.tile_pool(name="odram", bufs=1, space="DRAM") as odram,
            tc.tile_pool(name="psA", bufs=2, space="PSUM") as psA,
        ):
            # ---- resident tiles ----
            qhl = qkres.tile([P, H, S], BF16, tag="qhl")  # [qh; ql] per q-head
            khh = qkres.tile([P, H, S], BF16, tag="khh")  # [kh; kh] per k-head
            klB = qkres.tile([64, H, S], BF16, tag="klB")  # kl at base 0
            vmat = qkres.tile([P, NT, D], BF16, tag="vmat")  # [j_sub, j_tile, c]
            mposb = res.tile([P, NT, S], FP8, tag="mposb")  # [i_sub, i_tile, j]
            ones_row = res.tile([1, P], BF16, tag="ones")
            bqk_lo = res.tile([64, 16], F32, tag="bqk_lo")
            bqk_hi = res.tile([64, 16], F32, tag="bqk_hi")
            bv = res.tile([1, D], BF16, tag="bv")
            otm_d = odram.tile([P, NT, S], BF16, tag="otm_d")  # DRAM scratch

            nc.vector.memset(ones_row[:], 1.0)

            with tc.tile_pool(name="p12", bufs=1) as p12, \
                 tc.tile_pool(name="wstr", bufs=3) as wstr:
                xh = p12.tile([P, NT, S], BF16, tag="xh")  # [d_sub, d_tile, s]
                xl = p12.tile([P, NT, S], BF16, tag="xl")
                nc.sync.dma_start(xh[:], xh_d.ap().rearrange("(t p) s -> p t s", p=P))

                # ---------------- phase 1: V projection ----------------
                with tc.tile_pool(name="wvp", bufs=1) as wvp:
                    wv = wvp.tile([P, NT, D], BF16, tag="wv")
                    nc.sync.dma_start(
                        wv[:], wv_d.ap().rearrange("(t p) c -> p t c", p=P))
                    nc.sync.dma_start(
                        xl[:], xl_d.ap().rearrange("(t p) s -> p t s", p=P))
                    nc.sync.dma_start(bqk_lo[:], bqk_lo_d[:])
                    nc.sync.dma_start(bqk_hi[:], bqk_hi_d[:])
                    nc.sync.dma_start(bv[:], bv_d[:])
                    for st in range(NT):
                        ssl = slice(st * P, (st + 1) * P)
                        for nh in range(2):
                            hsl = slice(nh * 512, (nh + 1) * 512)
                            ps = psA.tile([P, 512], F32, tag="ps")
                            for k in range(NT):
                                nc.tensor.matmul(
                                    ps[:], xh[:, k, ssl], wv[:, k, hsl],
                                    start=(k == 0), stop=False)
                            nc.tensor.matmul(
                                ps[:], ones_row[:], bv[:, hsl],
                                start=False, stop=True)
                            nc.scalar.copy(vmat[:, st, hsl], ps[:])
                    nc.sync.dma_start(
                        mposb[:], m_d.ap().rearrange("(t p) j -> p t j", p=P))

                # ------- phase 2+3: interleaved q/k proj + attention -------
                def proj_et(et):
                    is_q = et < 8
                    wh = wstr.tile([P, NT, P], BF16, tag="wh")
                    wl = wstr.tile([P, NT, P], BF16, tag="wl")
                    esl = slice(et * P, (et + 1) * P)
                    nc.sync.dma_start(
                        wh[:], wqkh_d[:, esl].rearrange("(t p) e -> p t e", p=P))
                    nc.sync.dma_start(
                        wl[:], wqkl_d[:, esl].rearrange("(t p) e -> p t e", p=P))
                    hA = 2 * (et % 8)
                    hB = hA + 1
                    for nh in range(2):
                        hsl = slice(nh * 512, (nh + 1) * 512)
                        ps = psA.tile([P, 512], F32, tag="ps")
                        first = True
                        for k in range(NT):
                            for mi, (wt, xt) in enumerate(
                                    ((wh, xh), (wl, xh), (wh, xl))):
                                nc.tensor.matmul(
                                    ps[:], wt[:, k, :], xt[:, k, hsl],
                                    start=first,
                                    stop=(k == NT - 1 and mi == 2))
                                first = False
                        if is_q:
                            nc.scalar.activation(
                                qhl[0:64, hA, hsl], ps[0:64], IDENT,
                                bias=bqk_lo[:, et:et + 1])
                            nc.scalar.activation(
                                qhl[0:64, hB, hsl], ps[64:128], IDENT,
                                bias=bqk_hi[:, et:et + 1])
                            nc.vector.scalar_tensor_tensor(
                                out=qhl[64:128, hA, hsl], in0=ps[0:64],
                                scalar=bqk_lo[:, et:et + 1],
                                in1=qhl[0:64, hA, hsl], op0=ADD, op1=SUB)
                            nc.vector.scalar_tensor_tensor(
                                out=qhl[64:128, hB, hsl], in0=ps[64:128],
                                scalar=bqk_hi[:, et:et + 1],
                                in1=qhl[0:64, hB, hsl], op0=ADD, op1=SUB)
                        else:
                            nc.scalar.activation(
                                khh[0:64, hA, hsl], ps[0:64], IDENT,
                                bias=bqk_lo[:, et:et + 1])
                            nc.scalar.activation(
                                khh[64:128, hA, hsl], ps[0:64], IDENT,
                                bias=bqk_lo[:, et:et + 1])
                            nc.scalar.activation(
                                khh[0:64, hB, hsl], ps[64:128], IDENT,
                                bias=bqk_hi[:, et:et + 1])
                            nc.scalar.activation(
                                khh[64:128, hB, hsl], ps[64:128], IDENT,
                                bias=bqk_hi[:, et:et + 1])
                            nc.vector.scalar_tensor_tensor(
                                out=klB[:, hA, hsl], in0=ps[0:64],
                                scalar=bqk_lo[:, et:et + 1],
                                in1=khh[0:64, hA, hsl], op0=ADD, op1=SUB)
                            nc.vector.scalar_tensor_tensor(
                                out=klB[:, hB, hsl], in0=ps[64:128],
                                scalar=bqk_hi[:, et:et + 1],
                                in1=khh[0:64, hB, hsl], op0=ADD, op1=SUB)

                with tc.tile_pool(name="ppool", bufs=2) as ppool, \
                     tc.tile_pool(name="ptpool", bufs=2) as ptpool, \
                     tc.tile_pool(name="ostg", bufs=2) as ostg, \
                     tc.tile_pool(name="ps_s", bufs=2, space="PSUM") as ps_s, \
                     tc.tile_pool(name="ps_o", bufs=2, space="PSUM") as ps_o:
                    ptbs = {}

                    def attn_front(h):
                        ptb = ptpool.tile([P, NT, S], BF16, tag="ptb")
                        ptbs[h] = ptb
                        for it in range(NT):
                            isl = slice(it * P, (it + 1) * P)
                            pss = ps_s.tile([P, S], F32, tag="pss")
                            for nh in range(2):
                                hsl = slice(nh * 512, (nh + 1) * 512)
                                nc.tensor.matmul(
                                    pss[:, hsl], qhl[:, h, isl],
                                    khh[:, h, hsl], start=True, stop=False)
                                nc.tensor.matmul(
                                    pss[:, hsl], qhl[0:64, h, isl],
                                    klB[:, h, hsl], start=False, stop=True)
                            # mask-mult in place in PSUM (no SBUF ut tile)
                            nc.vector.scalar_tensor_tensor(
                                out=pss[:], in0=pss[:], scalar=SCALE,
                                in1=mposb[:, it, :], op0=MULT, op1=MULT)
                            umin = ppool.tile([P, 1], F32, tag="umin")
                            nc.vector.tensor_reduce(
                                out=umin[:], in_=pss[:], axis=AX, op=MIN)
                            pt = ppool.tile([P, S], BF16, tag="pt")
                            nc.scalar.activation(
                                out=pt[:], in_=pss[:], func=EXP,
                                bias=umin[:], scale=-1.0)
                            nc.sync.dma_start_transpose(ptb[:, :, isl], pt[:])

                    def attn_pv(h):
                        ptb = ptbs.pop(h)
                        csl = slice((h // 2) * P + (h % 2) * 64,
                                    (h // 2) * P + (h % 2) * 64 + 64)
                        obase = (h % 2) * 64
                        for nh in range(2):
                            hsl = slice(nh * 512, (nh + 1) * 512)
                            pso = ps_o.tile([64, 512], F32, tag="pso")
                            for jt in range(NT):
                                nc.tensor.matmul(
                                    pso[:], vmat[:, jt, csl], ptb[:, jt, hsl],
                                    start=(jt == 0), stop=(jt == NT - 1))
                            og = ostg.tile([64, 512], BF16, tag="og")
                            nc.scalar.copy(og[:], pso[:])
                            nc.sync.dma_start(
                                otm_d[obase:obase + 64, h // 2, hsl], og[:])

                    for i in range(8):
                        proj_et(i)
                        proj_et(8 + i)
                        attn_front(2 * i)
                        if i > 0:
                            attn_pv(2 * i - 1)
                        attn_front(2 * i + 1)
                        attn_pv(2 * i)
                    attn_pv(15)

            # ---------------- phase 4: output projection ----------------
            with tc.tile_pool(name="late", bufs=1) as late, \
                 tc.tile_pool(name="ypool", bufs=2) as ypool:
                wpt = late.tile([P, NT, D], BF16, tag="wp")
                bp = late.tile([1, D], BF16, tag="bp")
                otm = late.tile([P, NT, S], BF16, tag="otm")
                nc.sync.dma_start(
                    wpt[:], wp_d.ap().rearrange("(t p) d -> p t d", p=P))
                nc.sync.dma_start(bp[:], bp_d[:])
                nc.sync.dma_start(otm[:], otm_d[:])
                for st in range(NT):
                    ssl = slice(st * P, (st + 1) * P)
                    yt = ypool.tile([P, D], F32, tag="yt")
                    for nh in range(2):
                        hsl = slice(nh * 512, (nh + 1) * 512)
                        ps = psA.tile([P, 512], F32, tag="ps")
                        for ot in range(NT):
                            nc.tensor.matmul(
                                ps[:], otm[:, ot, ssl], wpt[:, ot, hsl],
                                start=(ot == 0), stop=False)
                        nc.tensor.matmul(
                            ps[:], ones_row[:], bp[:, hsl],
                            start=False, stop=True)
                        nc.scalar.copy(yt[:, hsl], ps[:])
                    nc.sync.dma_start(y_d[st * P:(st + 1) * P, :], yt[:])

    nc.compile()
    return nc


def _prep_inputs(x, mask, W_attn, b_attn, W_proj, b_proj):
    x = np.asarray(x, np.float32)
    mask = np.asarray(mask, np.float32)
    W_attn = np.asarray(W_attn, np.float32)
    b_attn = np.asarray(b_attn, np.float32).reshape(-1)
    W_proj = np.asarray(W_proj, np.float32)
    b_proj = np.asarray(b_proj, np.float32).reshape(-1)

    wqk = W_attn[:, : 2 * D]
    wqkh = _bf16(wqk)
    wqkl = _bf16(wqk - wqkh.astype(np.float32))
    wv = _bf16(W_attn[:, 2 * D:])
    wp = _bf16(W_proj)

    bqk = b_attn[: 2 * D].reshape(16, 128)  # [et, p]
    bqk_lo = np.ascontiguousarray(bqk[:, 0:64].T, np.float32)  # [64, 16]
    bqk_hi = np.ascontiguousarray(bqk[:, 64:128].T, np.float32)
    bv = _bf16(b_attn[2 * D:].reshape(1, D))
    bp = _bf16(b_proj.reshape(1, D))

    shared = dict(wqkh=wqkh, wqkl=wqkl, wv=wv, wp=wp,
                  bqk_lo=bqk_lo, bqk_hi=bqk_hi, bv=bv, bp=bp)
    in_maps = []
    for b in range(B):
        xT = np.ascontiguousarray(x[b].T)
        xh = _bf16(xT)
        xli = _bf16(xT - xh.astype(np.float32))
        in_maps.append(dict(
            xh=xh, xl=xli,
            m=np.ascontiguousarray(mask[b, 0].astype(ml_dtypes.float8_e4m3)),
            **shared))
    return in_maps


def kernel(x, mask, W_attn, b_attn, W_proj, b_proj, _trace=False, _trace_kwargs=None):
    if "nc" not in _CACHE:
        _CACHE["nc"] = _build()
    nc = _CACHE["nc"]
    in_maps = _prep_inputs(x, mask, W_attn, b_attn, W_proj, b_proj)
    kw = {}
    if _trace:
        kw = dict(trace=True, **(_trace_kwargs or {}))
    res = run_bass_kernel_spmd(nc, in_maps, core_ids=list(range(B)), **kw)
    out = np.stack([res.results[b]["y"] for b in range(B)], axis=0)
    if _trace:
        _CACHE["last_results"] = res
    return out


# revision 20
# speedup vs baseline: 1.1803x; 1.0266x over previous
"""Multi-head attention (degenerate multiplicative-mask softmax) on 8 TRN2 cores.

Sharding: pure data-parallel over batch (B=8 -> 1 batch element per core).
No collectives. Each core computes its batch's full attention + output proj.

v3 design (v1 757us -> v2 623us -> v3):
  - Scores: 3-pass bf16 hi/lo packed into 2 matmuls: K=128 [qh;ql]x[kh;kh]
    + K=64 qh x kl. Same numerics as v1, 2/3 the PE time.
  - P^T built by DMA xbar transposes (dma_start_transpose, 3D out) on the
    otherwise-idle DMA engines - no PE transposes, no psum->sbuf copies.
  - FULL proj/attention interleave: after each (q-et, k-et) projection pair,
    the two ready heads' attention is emitted, so vector/scalar/DMA attention
    work overlaps the projection matmul stream and TensorE becomes the only
    wall. SBUF fits because: masked scores live in PSUM (no ut tile), the
    attention output otm round-trips through a DRAM tile (saves 16K/part),
    mask is fp8, and phase-4 weights load into a late-scoped pool.
  - PV runs one head behind the scores pipeline (ptb double-buffered), so
    the in-order PE never stalls on the xbar transposes.
  - Biases folded: q/k exact-f32 via activation(Identity, bias) on the
    psum->sbuf copy; v/proj via single K=1 ones-matmul.

Precision: identical hi/lo 3-pass scheme as v1 (rel err 0.0054, all from the
single-bf16 V path; argmax flips ~0).
"""
import sys

sys.path.insert(0, "/opt/trn_rl_repo")

import numpy as np
import ml_dtypes

import concourse.bass as bass  # noqa: F401
import concourse.tile as tile
from concourse import bacc, mybir
from concourse.bass_utils import run_bass_kernel_spmd

F32 = mybir.dt.float32
FP8 = mybir.dt.float8e4
BF16 = mybir.dt.bfloat16
ADD = mybir.AluOpType.add
SUB = mybir.AluOpType.subtract
MULT = mybir.AluOpType.mult
MIN = mybir.AluOpType.min
IDENT = mybir.ActivationFunctionType.Identity
EXP = mybir.ActivationFunctionType.Exp
AX = mybir.AxisListType.X

B, S, D = 8, 1024, 1024
H, DH = 16, 64
P = 128
NT = S // P
SCALE = 1.25e8  # 1e9 / 8

_CACHE = {}


def _bf16(a):
    return np.ascontiguousarray(a.astype(ml_dtypes.bfloat16))


def _build():
    nc = bacc.Bacc(None)

    xh_d = nc.dram_tensor("xh", [D, S], BF16, kind="ExternalInput")  # x[b].T hi
    xl_d = nc.dram_tensor("xl", [D, S], BF16, kind="ExternalInput")  # x[b].T lo
    m_d = nc.dram_tensor("m", [S, S], FP8, kind="ExternalInput")  # mask [i, j]
    wqkh_d = nc.dram_tensor("wqkh", [D, 2 * D], BF16, kind="ExternalInput")
    wqkl_d = nc.dram_tensor("wqkl", [D, 2 * D], BF16, kind="ExternalInput")
    wv_d = nc.dram_tensor("wv", [D, D], BF16, kind="ExternalInput")
    wp_d = nc.dram_tensor("wp", [D, D], BF16, kind="ExternalInput")
    bqk_lo_d = nc.dram_tensor("bqk_lo", [64, 16], F32, kind="ExternalInput")
    bqk_hi_d = nc.dram_tensor("bqk_hi", [64, 16], F32, kind="ExternalInput")
    bv_d = nc.dram_tensor("bv", [1, D], BF16, kind="ExternalInput")
    bp_d = nc.dram_tensor("bp", [1, D], BF16, kind="ExternalInput")
    y_d = nc.dram_tensor("y", [S, D], F32, kind="ExternalOutput")

    with tile.TileContext(nc) as tc:
        with (
            tc.tile_pool(name="res", bufs=1) as res,
            tc.tile_pool(name="qkres", bufs=1) as qkres,
            tc.tile_pool(name="odram", bufs=1, space="DRAM") as odram,
            tc.tile_pool(name="psA", bufs=2, space="PSUM") as psA,
        ):
            # ---- resident tiles ----
            qhl = qkres.tile([P, H, S], BF16, tag="qhl")  # [qh; ql] per q-head
            khh = qkres.tile([P, H, S], BF16, tag="khh")  # [kh; kh] per k-head
            klB = qkres.tile([64, H, S], BF16, tag="klB")  # kl at base 0
            vmat = qkres.tile([P, NT, D], BF16, tag="vmat")  # [j_sub, j_tile, c]
            mposb = res.tile([P, NT, S], FP8, tag="mposb")  # [i_sub, i_tile, j]
            ones_row = res.tile([1, P], BF16, tag="ones")
            bqk_lo = res.tile([64, 16], F32, tag="bqk_lo")
            bqk_hi = res.tile([64, 16], F32, tag="bqk_hi")
            otm_d = odram.tile([P, NT, S], BF16, tag="otm_d")  # DRAM scratch

            nc.vector.memset(ones_row[:], 1.0)

            with tc.tile_pool(name="p12", bufs=1) as p12, \
                 tc.tile_pool(name="wstr", bufs=2) as wstr:
                xh = p12.tile([P, NT, S], BF16, tag="xh")  # [d_sub, d_tile, s]
                xl = p12.tile([P, NT, S], BF16, tag="xl")
                xh_r = xh_d.ap().rearrange("(t p) s -> p t s", p=P)
                # split startup DMAs so the first V matmuls start sooner
                nc.sync.dma_start(xh[:, 0:4, :], xh_r[:, 0:4, :])

                # ---------------- phase 1: V projection ----------------
                with tc.tile_pool(name="wvp", bufs=1) as wvp:
                    wv = wvp.tile([P, NT, D], BF16, tag="wv")
                    bv = wvp.tile([1, D], BF16, tag="bv")
                    wv_r = wv_d.ap().rearrange("(t p) c -> p t c", p=P)
                    nc.sync.dma_start(wv[:, 0:4, :], wv_r[:, 0:4, :])
                    nc.sync.dma_start(bv[:], bv_d[:])
                    nc.sync.dma_start(xh[:, 4:8, :], xh_r[:, 4:8, :])
                    nc.sync.dma_start(wv[:, 4:8, :], wv_r[:, 4:8, :])
                    nc.sync.dma_start(
                        xl[:], xl_d.ap().rearrange("(t p) s -> p t s", p=P))
                    nc.sync.dma_start(bqk_lo[:], bqk_lo_d[:])
                    nc.sync.dma_start(bqk_hi[:], bqk_hi_d[:])
                    for st in range(NT):
                        ssl = slice(st * P, (st + 1) * P)
                        for nh in range(2):
                            hsl = slice(nh * 512, (nh + 1) * 512)
                            ps = psA.tile([P, 512], F32, tag="ps")
                            for k in range(NT):
                                nc.tensor.matmul(
                                    ps[:], xh[:, k, ssl], wv[:, k, hsl],
                                    start=(k == 0), stop=False)
                            nc.tensor.matmul(
                                ps[:], ones_row[:], bv[:, hsl],
                                start=False, stop=True)
                            nc.scalar.copy(vmat[:, st, hsl], ps[:])
                    nc.sync.dma_start(
                        mposb[:], m_d.ap().rearrange("(t p) j -> p t j", p=P))

                # ------- phase 2+3: interleaved q/k proj + attention -------
                def proj_et(et):
                    is_q = et < 8
                    wh = wstr.tile([P, NT, P], BF16, tag="wh")
                    wl = wstr.tile([P, NT, P], BF16, tag="wl")
                    esl = slice(et * P, (et + 1) * P)
                    nc.sync.dma_start(
                        wh[:], wqkh_d[:, esl].rearrange("(t p) e -> p t e", p=P))
                    nc.sync.dma_start(
                        wl[:], wqkl_d[:, esl].rearrange("(t p) e -> p t e", p=P))
                    hA = 2 * (et % 8)
                    hB = hA + 1
                    for nh in range(2):
                        hsl = slice(nh * 512, (nh + 1) * 512)
                        ps = psA.tile([P, 512], F32, tag="ps")
                        first = True
                        for k in range(NT):
                            for mi, (wt, xt) in enumerate(
                                    ((wh, xh), (wl, xh), (wh, xl))):
                                nc.tensor.matmul(
                                    ps[:], wt[:, k, :], xt[:, k, hsl],
                                    start=first,
                                    stop=(k == NT - 1 and mi == 2))
                                first = False
                        if is_q:
                            nc.scalar.activation(
                                qhl[0:64, hA, hsl], ps[0:64], IDENT,
                                bias=bqk_lo[:, et:et + 1])
                            nc.scalar.activation(
                                qhl[0:64, hB, hsl], ps[64:128], IDENT,
                                bias=bqk_hi[:, et:et + 1])
                            nc.vector.scalar_tensor_tensor(
                                out=qhl[64:128, hA, hsl], in0=ps[0:64],
                                scalar=bqk_lo[:, et:et + 1],
                                in1=qhl[0:64, hA, hsl], op0=ADD, op1=SUB)
                            nc.vector.scalar_tensor_tensor(
                                out=qhl[64:128, hB, hsl], in0=ps[64:128],
                                scalar=bqk_hi[:, et:et + 1],
                                in1=qhl[0:64, hB, hsl], op0=ADD, op1=SUB)
                        else:
                            nc.scalar.activation(
                                khh[0:64, hA, hsl], ps[0:64], IDENT,
                                bias=bqk_lo[:, et:et + 1])
                            nc.scalar.activation(
                                khh[64:128, hA, hsl], ps[0:64], IDENT,
                                bias=bqk_lo[:, et:et + 1])
                            nc.scalar.activation(
                                khh[0:64, hB, hsl], ps[64:128], IDENT,
                                bias=bqk_hi[:, et:et + 1])
                            nc.scalar.activation(
                                khh[64:128, hB, hsl], ps[64:128], IDENT,
                                bias=bqk_hi[:, et:et + 1])
                            nc.vector.scalar_tensor_tensor(
                                out=klB[:, hA, hsl], in0=ps[0:64],
                                scalar=bqk_lo[:, et:et + 1],
                                in1=khh[0:64, hA, hsl], op0=ADD, op1=SUB)
                            nc.vector.scalar_tensor_tensor(
                                out=klB[:, hB, hsl], in0=ps[64:128],
                                scalar=bqk_hi[:, et:et + 1],
                                in1=khh[0:64, hB, hsl], op0=ADD, op1=SUB)

                with tc.tile_pool(name="ppool", bufs=2) as ppool, \
                     tc.tile_pool(name="ptpool", bufs=2) as ptpool, \
                     tc.tile_pool(name="ostg", bufs=2) as ostg, \
                     tc.tile_pool(name="ps_s", bufs=2, space="PSUM") as ps_s, \
                     tc.tile_pool(name="ps_o", bufs=2, space="PSUM") as ps_o:
                    ptbs = {}

                    def attn_front(h):
                        ptb = ptpool.tile([P, NT, S], BF16, tag="ptb")
                        ptbs[h] = ptb
                        for it in range(NT):
                            isl = slice(it * P, (it + 1) * P)
                            pss = ps_s.tile([P, S], F32, tag="pss")
                            for nh in range(2):
                                hsl = slice(nh * 512, (nh + 1) * 512)
                                nc.tensor.matmul(
                                    pss[:, hsl], qhl[:, h, isl],
                                    khh[:, h, hsl], start=True, stop=False)
                                nc.tensor.matmul(
                                    pss[:, hsl], qhl[0:64, h, isl],
                                    klB[:, h, hsl], start=False, stop=True)
                            # mask-mult to SBUF: frees the psum tile fast so
                            # the PE can run ahead (keeps the pstate ramped)
                            ut = ppool.tile([P, S], F32, tag="ut")
                            nc.vector.scalar_tensor_tensor(
                                out=ut[:], in0=pss[:], scalar=SCALE,
                                in1=mposb[:, it, :], op0=MULT, op1=MULT)
                            umin = ppool.tile([P, 1], F32, tag="umin")
                            nc.vector.tensor_reduce(
                                out=umin[:], in_=ut[:], axis=AX, op=MIN)
                            pt = ppool.tile([P, S], BF16, tag="pt")
                            nc.scalar.activation(
                                out=pt[:], in_=ut[:], func=EXP,
                                bias=umin[:], scale=-1.0)
                            nc.sync.dma_start_transpose(ptb[:, :, isl], pt[:])

                    def attn_pv(h):
                        ptb = ptbs.pop(h)
                        csl = slice((h // 2) * P + (h % 2) * 64,
                                    (h // 2) * P + (h % 2) * 64 + 64)
                        obase = (h % 2) * 64
                        for nh in range(2):
                            hsl = slice(nh * 512, (nh + 1) * 512)
                            pso = ps_o.tile([64, 512], F32, tag="pso")
                            for jt in range(NT):
                                nc.tensor.matmul(
                                    pso[:], vmat[:, jt, csl], ptb[:, jt, hsl],
                                    start=(jt == 0), stop=(jt == NT - 1))
                            og = ostg.tile([64, 512], BF16, tag="og")
                            nc.scalar.copy(og[:], pso[:])
                            nc.sync.dma_start(
                                otm_d[obase:obase + 64, h // 2, hsl], og[:])

                    for i in range(8):
                        proj_et(i)
                        proj_et(8 + i)
                        attn_front(2 * i)
                        if i > 0:
                            attn_pv(2 * i - 1)
                        attn_front(2 * i + 1)
                        attn_pv(2 * i)
                    attn_pv(15)

            # ---------------- phase 4: output projection ----------------
            with tc.tile_pool(name="late", bufs=1) as late, \
                 tc.tile_pool(name="ypool", bufs=2) as ypool:
                wpt = late.tile([P, NT, D], BF16, tag="wp")
                bp = late.tile([1, D], BF16, tag="bp")
                otm = late.tile([P, NT, S], BF16, tag="otm")
                nc.sync.dma_start(
                    wpt[:], wp_d.ap().rearrange("(t p) d -> p t d", p=P))
                nc.sync.dma_start(bp[:], bp_d[:])
                # per-ot reload: each c-tile only waits for its two heads' PV
                for ot in range(NT):
                    nc.sync.dma_start(otm[:, ot, :], otm_d[:, ot, :])
                for st in range(NT):
                    ssl = slice(st * P, (st + 1) * P)
                    yt = ypool.tile([P, D], F32, tag="yt")
                    for nh in range(2):
                        hsl = slice(nh * 512, (nh + 1) * 512)
                        ps = psA.tile([P, 512], F32, tag="ps")
                        for ot in range(NT):
                            nc.tensor.matmul(
                                ps[:], otm[:, ot, ssl], wpt[:, ot, hsl],
                                start=(ot == 0), stop=False)
                        nc.tensor.matmul(
                            ps[:], ones_row[:], bp[:, hsl],
                            start=False, stop=True)
                        nc.scalar.copy(yt[:, hsl], ps[:])
                    nc.sync.dma_start(y_d[st * P:(st + 1) * P, :], yt[:])

    nc.compile()
    return nc


def _prep_inputs(x, mask, W_attn, b_attn, W_proj, b_proj):
    x = np.asarray(x, np.float32)
    mask = np.asarray(mask, np.float32)
    W_attn = np.asarray(W_attn, np.float32)
    b_attn = np.asarray(b_attn, np.float32).reshape(-1)
    W_proj = np.asarray(W_proj, np.float32)
    b_proj = np.asarray(b_proj, np.float32).reshape(-1)

    wqk = W_attn[:, : 2 * D]
    wqkh = _bf16(wqk)
    wqkl = _bf16(wqk - wqkh.astype(np.float32))
    wv = _bf16(W_attn[:, 2 * D:])
    wp = _bf16(W_proj)

    bqk = b_attn[: 2 * D].reshape(16, 128)  # [et, p]
    bqk_lo = np.ascontiguousarray(bqk[:, 0:64].T, np.float32)  # [64, 16]
    bqk_hi = np.ascontiguousarray(bqk[:, 64:128].T, np.float32)
    bv = _bf16(b_attn[2 * D:].reshape(1, D))
    bp = _bf16(b_proj.reshape(1, D))

    shared = dict(wqkh=wqkh, wqkl=wqkl, wv=wv, wp=wp,
                  bqk_lo=bqk_lo, bqk_hi=bqk_hi, bv=bv, bp=bp)
    in_maps = []
    for b in range(B):
        xT = np.ascontiguousarray(x[b].T)
        xh = _bf16(xT)
        xli = _bf16(xT - xh.astype(np.float32))
        in_maps.append(dict(
            xh=xh, xl=xli,
            m=np.ascontiguousarray(mask[b, 0].astype(ml_dtypes.float8_e4m3)),
            **shared))
    return in_maps


def kernel(x, mask, W_attn, b_attn, W_proj, b_proj, _trace=False, _trace_kwargs=None):
    if "nc" not in _CACHE:
        _CACHE["nc"] = _build()
    nc = _CACHE["nc"]
    in_maps = _prep_inputs(x, mask, W_attn, b_attn, W_proj, b_proj)
    kw = {}
    if _trace:
        kw = dict(trace=True, **(_trace_kwargs or {}))
    res = run_bass_kernel_spmd(nc, in_maps, core_ids=list(range(B)), **kw)
    out = np.stack([res.results[b]["y"] for b in range(B)], axis=0)
    if _trace:
        _CACHE["last_results"] = res
    return out


# revision 21
# speedup vs baseline: 1.1898x; 1.0080x over previous
"""Multi-head attention (degenerate multiplicative-mask softmax) on 8 TRN2 cores.

Sharding: pure data-parallel over batch (B=8 -> 1 batch element per core).
No collectives. Each core computes its batch's full attention + output proj.

v3 design (v1 757us -> v2 623us -> v3):
  - Scores: 3-pass bf16 hi/lo packed into 2 matmuls: K=128 [qh;ql]x[kh;kh]
    + K=64 qh x kl. Same numerics as v1, 2/3 the PE time.
  - P^T built by DMA xbar transposes (dma_start_transpose, 3D out) on the
    otherwise-idle DMA engines - no PE transposes, no psum->sbuf copies.
  - FULL proj/attention interleave: after each (q-et, k-et) projection pair,
    the two ready heads' attention is emitted, so vector/scalar/DMA attention
    work overlaps the projection matmul stream and TensorE becomes the only
    wall. SBUF fits because: masked scores live in PSUM (no ut tile), the
    attention output otm round-trips through a DRAM tile (saves 16K/part),
    mask is fp8, and phase-4 weights load into a late-scoped pool.
  - PV runs one head behind the scores pipeline (ptb double-buffered), so
    the in-order PE never stalls on the xbar transposes.
  - Biases folded: q/k exact-f32 via activation(Identity, bias) on the
    psum->sbuf copy; v/proj via single K=1 ones-matmul.

Precision: identical hi/lo 3-pass scheme as v1 (rel err 0.0054, all from the
single-bf16 V path; argmax flips ~0).
"""
import sys

sys.path.insert(0, "/opt/trn_rl_repo")

import numpy as np
import ml_dtypes

import concourse.bass as bass  # noqa: F401
import concourse.tile as tile
from concourse import bacc, mybir
from concourse.bass_utils import run_bass_kernel_spmd

F32 = mybir.dt.float32
FP8 = mybir.dt.float8e4
BF16 = mybir.dt.bfloat16
ADD = mybir.AluOpType.add
SUB = mybir.AluOpType.subtract
MULT = mybir.AluOpType.mult
MIN = mybir.AluOpType.min
IDENT = mybir.ActivationFunctionType.Identity
EXP = mybir.ActivationFunctionType.Exp
AX = mybir.AxisListType.X

B, S, D = 8, 1024, 1024
H, DH = 16, 64
P = 128
NT = S // P
SCALE = 1.25e8  # 1e9 / 8

_CACHE = {}


def _bf16(a):
    return np.ascontiguousarray(a.astype(ml_dtypes.bfloat16))


def _build():
    nc = bacc.Bacc(None)

    xh_d = nc.dram_tensor("xh", [D, S], BF16, kind="ExternalInput")  # x[b].T hi
    xl_d = nc.dram_tensor("xl", [D, S], BF16, kind="ExternalInput")  # x[b].T lo
    m_d = nc.dram_tensor("m", [S, S], FP8, kind="ExternalInput")  # mask [i, j]
    wqkh_d = nc.dram_tensor("wqkh", [D, 2 * D], BF16, kind="ExternalInput")
    wqkl_d = nc.dram_tensor("wqkl", [D, 2 * D], BF16, kind="ExternalInput")
    wv_d = nc.dram_tensor("wv", [D, D], BF16, kind="ExternalInput")
    wp_d = nc.dram_tensor("wp", [D, D], BF16, kind="ExternalInput")
    bqk_lo_d = nc.dram_tensor("bqk_lo", [64, 16], F32, kind="ExternalInput")
    bqk_hi_d = nc.dram_tensor("bqk_hi", [64, 16], F32, kind="ExternalInput")
    bv_d = nc.dram_tensor("bv", [1, D], BF16, kind="ExternalInput")
    bp_d = nc.dram_tensor("bp", [1, D], BF16, kind="ExternalInput")
    y_d = nc.dram_tensor("y", [S, D], F32, kind="ExternalOutput")

    with tile.TileContext(nc) as tc:
        with (
            tc.tile_pool(name="res", bufs=1) as res,
            tc.tile_pool(name="qkres", bufs=1) as qkres,
            tc.tile_pool(name="odram", bufs=1, space="DRAM") as odram,
            tc.tile_pool(name="psA", bufs=2, space="PSUM") as psA,
        ):
            # ---- resident tiles ----
            qhl = qkres.tile([P, H, S], BF16, tag="qhl")  # [qh; ql] per q-head
            khh = qkres.tile([P, H, S], BF16, tag="khh")  # [kh; kh] per k-head
            klB = qkres.tile([P, H, S], BF16, tag="klB")  # [kl; kl] dup
            vmat = qkres.tile([P, NT, D], BF16, tag="vmat")  # [j_sub, j_tile, c]
            mposb = res.tile([P, NT, S], FP8, tag="mposb")  # [i_sub, i_tile, j]
            ones_row = res.tile([1, P], BF16, tag="ones")
            bqk_lo = res.tile([64, 16], F32, tag="bqk_lo")
            bqk_hi = res.tile([64, 16], F32, tag="bqk_hi")
            otm_d = odram.tile([P, NT, S], BF16, tag="otm_d")  # DRAM scratch

            nc.vector.memset(ones_row[:], 1.0)

            with tc.tile_pool(name="p12", bufs=1) as p12, \
                 tc.tile_pool(name="wstr", bufs=2) as wstr:
                xh = p12.tile([P, NT, S], BF16, tag="xh")  # [d_sub, d_tile, s]
                xl = p12.tile([P, NT, S], BF16, tag="xl")
                xh_r = xh_d.ap().rearrange("(t p) s -> p t s", p=P)
                # split startup DMAs so the first V matmuls start sooner
                nc.sync.dma_start(xh[:, 0:4, :], xh_r[:, 0:4, :])

                # ---------------- phase 1: V projection ----------------
                with tc.tile_pool(name="wvp", bufs=1) as wvp:
                    wv = wvp.tile([P, NT, D], BF16, tag="wv")
                    bv = wvp.tile([1, D], BF16, tag="bv")
                    wv_r = wv_d.ap().rearrange("(t p) c -> p t c", p=P)
                    nc.sync.dma_start(wv[:, 0:4, :], wv_r[:, 0:4, :])
                    nc.sync.dma_start(bv[:], bv_d[:])
                    nc.sync.dma_start(xh[:, 4:8, :], xh_r[:, 4:8, :])
                    nc.sync.dma_start(wv[:, 4:8, :], wv_r[:, 4:8, :])
                    nc.sync.dma_start(
                        xl[:], xl_d.ap().rearrange("(t p) s -> p t s", p=P))
                    nc.sync.dma_start(bqk_lo[:], bqk_lo_d[:])
                    nc.sync.dma_start(bqk_hi[:], bqk_hi_d[:])
                    for st in range(NT):
                        ssl = slice(st * P, (st + 1) * P)
                        for nh in range(2):
                            hsl = slice(nh * 512, (nh + 1) * 512)
                            ps = psA.tile([P, 512], F32, tag="ps")
                            for k in range(NT):
                                nc.tensor.matmul(
                                    ps[:], xh[:, k, ssl], wv[:, k, hsl],
                                    start=(k == 0), stop=False)
                            nc.tensor.matmul(
                                ps[:], ones_row[:], bv[:, hsl],
                                start=False, stop=True)
                            nc.scalar.copy(vmat[:, st, hsl], ps[:])
                    nc.sync.dma_start(
                        mposb[:], m_d.ap().rearrange("(t p) j -> p t j", p=P))

                # ------- phase 2+3: interleaved q/k proj + attention -------
                def proj_et(et):
                    is_q = et < 8
                    wh = wstr.tile([P, NT, P], BF16, tag="wh")
                    wl = wstr.tile([P, NT, P], BF16, tag="wl")
                    esl = slice(et * P, (et + 1) * P)
                    nc.sync.dma_start(
                        wh[:], wqkh_d[:, esl].rearrange("(t p) e -> p t e", p=P))
                    nc.sync.dma_start(
                        wl[:], wqkl_d[:, esl].rearrange("(t p) e -> p t e", p=P))
                    hA = 2 * (et % 8)
                    hB = hA + 1
                    for nh in range(2):
                        hsl = slice(nh * 512, (nh + 1) * 512)
                        ps = psA.tile([P, 512], F32, tag="ps")
                        first = True
                        for k in range(NT):
                            for mi, (wt, xt) in enumerate(
                                    ((wh, xh), (wl, xh), (wh, xl))):
                                nc.tensor.matmul(
                                    ps[:], wt[:, k, :], xt[:, k, hsl],
                                    start=first,
                                    stop=(k == NT - 1 and mi == 2))
                                first = False
                        if is_q:
                            nc.scalar.activation(
                                qhl[0:64, hA, hsl], ps[0:64], IDENT,
                                bias=bqk_lo[:, et:et + 1])
                            nc.scalar.activation(
                                qhl[0:64, hB, hsl], ps[64:128], IDENT,
                                bias=bqk_hi[:, et:et + 1])
                            nc.vector.scalar_tensor_tensor(
                                out=qhl[64:128, hA, hsl], in0=ps[0:64],
                                scalar=bqk_lo[:, et:et + 1],
                                in1=qhl[0:64, hA, hsl], op0=ADD, op1=SUB)
                            nc.vector.scalar_tensor_tensor(
                                out=qhl[64:128, hB, hsl], in0=ps[64:128],
                                scalar=bqk_hi[:, et:et + 1],
                                in1=qhl[0:64, hB, hsl], op0=ADD, op1=SUB)
                        else:
                            nc.scalar.activation(
                                khh[0:64, hA, hsl], ps[0:64], IDENT,
                                bias=bqk_lo[:, et:et + 1])
                            nc.scalar.activation(
                                khh[64:128, hA, hsl], ps[0:64], IDENT,
                                bias=bqk_lo[:, et:et + 1])
                            nc.scalar.activation(
                                khh[0:64, hB, hsl], ps[64:128], IDENT,
                                bias=bqk_hi[:, et:et + 1])
                            nc.scalar.activation(
                                khh[64:128, hB, hsl], ps[64:128], IDENT,
                                bias=bqk_hi[:, et:et + 1])
                            nc.vector.scalar_tensor_tensor(
                                out=klB[0:64, hA, hsl], in0=ps[0:64],
                                scalar=bqk_lo[:, et:et + 1],
                                in1=khh[0:64, hA, hsl], op0=ADD, op1=SUB)
                            nc.vector.scalar_tensor_tensor(
                                out=klB[0:64, hB, hsl], in0=ps[64:128],
                                scalar=bqk_hi[:, et:et + 1],
                                in1=khh[0:64, hB, hsl], op0=ADD, op1=SUB)
                            nc.scalar.copy(
                                klB[64:128, hA, hsl], klB[0:64, hA, hsl])
                            nc.scalar.copy(
                                klB[64:128, hB, hsl], klB[0:64, hB, hsl])

                with tc.tile_pool(name="ppool", bufs=2) as ppool, \
                     tc.tile_pool(name="ptpool", bufs=2) as ptpool, \
                     tc.tile_pool(name="ostg", bufs=2) as ostg, \
                     tc.tile_pool(name="ps_s", bufs=2, space="PSUM") as ps_s, \
                     tc.tile_pool(name="ps_o", bufs=2, space="PSUM") as ps_o:
                    ptbs = {}

                    def attn_front(h):
                        ptb = ptpool.tile([P, NT, S], BF16, tag="ptb")
                        ptbs[h] = ptb
                        for it in range(NT):
                            isl = slice(it * P, (it + 1) * P)
                            pss = ps_s.tile([P, S], F32, tag="pss")
                            for nh in range(2):
                                hsl = slice(nh * 512, (nh + 1) * 512)
                                nc.tensor.matmul(
                                    pss[:, hsl], qhl[:, h, isl],
                                    khh[:, h, hsl], start=True, stop=False)
                                nc.tensor.matmul(
                                    pss[:, hsl], qhl[:, h, isl],
                                    klB[:, h, hsl], start=False, stop=True)
                            # mask-mult to SBUF: frees the psum tile fast so
                            # the PE can run ahead (keeps the pstate ramped)
                            ut = ppool.tile([P, S], F32, tag="ut")
                            nc.vector.scalar_tensor_tensor(
                                out=ut[:], in0=pss[:], scalar=SCALE,
                                in1=mposb[:, it, :], op0=MULT, op1=MULT)
                            umin = ppool.tile([P, 1], F32, tag="umin")
                            nc.vector.tensor_reduce(
                                out=umin[:], in_=ut[:], axis=AX, op=MIN)
                            pt = ppool.tile([P, S], BF16, tag="pt")
                            nc.scalar.activation(
                                out=pt[:], in_=ut[:], func=EXP,
                                bias=umin[:], scale=-1.0)
                            nc.sync.dma_start_transpose(ptb[:, :, isl], pt[:])

                    def attn_pv(h):
                        ptb = ptbs.pop(h)
                        csl = slice((h // 2) * P + (h % 2) * 64,
                                    (h // 2) * P + (h % 2) * 64 + 64)
                        obase = (h % 2) * 64
                        for nh in range(2):
                            hsl = slice(nh * 512, (nh + 1) * 512)
                            pso = ps_o.tile([64, 512], F32, tag="pso")
                            for jt in range(NT):
                                nc.tensor.matmul(
                                    pso[:], vmat[:, jt, csl], ptb[:, jt, hsl],
                                    start=(jt == 0), stop=(jt == NT - 1))
                            og = ostg.tile([64, 512], BF16, tag="og")
                            nc.scalar.copy(og[:], pso[:])
                            nc.sync.dma_start(
                                otm_d[obase:obase + 64, h // 2, hsl], og[:])

                    for i in range(8):
                        proj_et(i)
                        proj_et(8 + i)
                        attn_front(2 * i)
                        if i > 0:
                            attn_pv(2 * i - 1)
                        attn_front(2 * i + 1)
                        attn_pv(2 * i)
                    attn_pv(15)

            # ---------------- phase 4: output projection ----------------
            with tc.tile_pool(name="late", bufs=1) as late, \
                 tc.tile_pool(name="ypool", bufs=2) as ypool:
                wpt = late.tile([P, NT, D], BF16, tag="wp")
                bp = late.tile([1, D], BF16, tag="bp")
                otm = late.tile([P, NT, S], BF16, tag="otm")
                nc.sync.dma_start(
                    wpt[:], wp_d.ap().rearrange("(t p) d -> p t d", p=P))
                nc.sync.dma_start(bp[:], bp_d[:])
                # per-ot reload: each c-tile only waits for its two heads' PV
                for ot in range(NT):
                    nc.sync.dma_start(otm[:, ot, :], otm_d[:, ot, :])
                for st in range(NT):
                    ssl = slice(st * P, (st + 1) * P)
                    yt = ypool.tile([P, D], F32, tag="yt")
                    for nh in range(2):
                        hsl = slice(nh * 512, (nh + 1) * 512)
                        ps = psA.tile([P, 512], F32, tag="ps")
                        for ot in range(NT):
                            nc.tensor.matmul(
                                ps[:], otm[:, ot, ssl], wpt[:, ot, hsl],
                                start=(ot == 0), stop=False)
                        nc.tensor.matmul(
                            ps[:], ones_row[:], bp[:, hsl],
                            start=False, stop=True)
                        nc.scalar.copy(yt[:, hsl], ps[:])
                    nc.sync.dma_start(y_d[st * P:(st + 1) * P, :], yt[:])

    nc.compile()
    return nc


def _prep_inputs(x, mask, W_attn, b_attn, W_proj, b_proj):
    x = np.asarray(x, np.float32)
    mask = np.asarray(mask, np.float32)
    W_attn = np.asarray(W_attn, np.float32)
    b_attn = np.asarray(b_attn, np.float32).reshape(-1)
    W_proj = np.asarray(W_proj, np.float32)
    b_proj = np.asarray(b_proj, np.float32).reshape(-1)

    wqk = W_attn[:, : 2 * D]
    wqkh = _bf16(wqk)
    wqkl = _bf16(wqk - wqkh.astype(np.float32))
    wv = _bf16(W_attn[:, 2 * D:])
    wp = _bf16(W_proj)

    bqk = b_attn[: 2 * D].reshape(16, 128)  # [et, p]
    bqk_lo = np.ascontiguousarray(bqk[:, 0:64].T, np.float32)  # [64, 16]
    bqk_hi = np.ascontiguousarray(bqk[:, 64:128].T, np.float32)
    bv = _bf16(b_attn[2 * D:].reshape(1, D))
    bp = _bf16(b_proj.reshape(1, D))

    shared = dict(wqkh=wqkh, wqkl=wqkl, wv=wv, wp=wp,
                  bqk_lo=bqk_lo, bqk_hi=bqk_hi, bv=bv, bp=bp)
    in_maps = []
    for b in range(B):
        xT = np.ascontiguousarray(x[b].T)
        xh = _bf16(xT)
        xli = _bf16(xT - xh.astype(np.float32))
        in_maps.append(dict(
            xh=xh, xl=xli,
            m=np.ascontiguousarray(mask[b, 0].astype(ml_dtypes.float8_e4m3)),
            **shared))
    return in_maps


def kernel(x, mask, W_attn, b_attn, W_proj, b_proj, _trace=False, _trace_kwargs=None):
    if "nc" not in _CACHE:
        _CACHE["nc"] = _build()
    nc = _CACHE["nc"]
    in_maps = _prep_inputs(x, mask, W_attn, b_attn, W_proj, b_proj)
    kw = {}
    if _trace:
        kw = dict(trace=True, **(_trace_kwargs or {}))
    res = run_bass_kernel_spmd(nc, in_maps, core_ids=list(range(B)), **kw)
    out = np.stack([res.results[b]["y"] for b in range(B)], axis=0)
    if _trace:
        _CACHE["last_results"] = res
    return out


# revision 22
# speedup vs baseline: 1.2027x; 1.0108x over previous
"""Multi-head attention (degenerate multiplicative-mask softmax) on 8 TRN2 cores.

Sharding: pure data-parallel over batch (B=8 -> 1 batch element per core).
No collectives. Each core computes its batch's full attention + output proj.

v3 design (v1 757us -> v2 623us -> v3):
  - Scores: 3-pass bf16 hi/lo packed into 2 matmuls: K=128 [qh;ql]x[kh;kh]
    + K=64 qh x kl. Same numerics as v1, 2/3 the PE time.
  - P^T built by DMA xbar transposes (dma_start_transpose, 3D out) on the
    otherwise-idle DMA engines - no PE transposes, no psum->sbuf copies.
  - FULL proj/attention interleave: after each (q-et, k-et) projection pair,
    the two ready heads' attention is emitted, so vector/scalar/DMA attention
    work overlaps the projection matmul stream and TensorE becomes the only
    wall. SBUF fits because: masked scores live in PSUM (no ut tile), the
    attention output otm round-trips through a DRAM tile (saves 16K/part),
    mask is fp8, and phase-4 weights load into a late-scoped pool.
  - PV runs one head behind the scores pipeline (ptb double-buffered), so
    the in-order PE never stalls on the xbar transposes.
  - Biases folded: q/k exact-f32 via activation(Identity, bias) on the
    psum->sbuf copy; v/proj via single K=1 ones-matmul.

Precision: identical hi/lo 3-pass scheme as v1 (rel err 0.0054, all from the
single-bf16 V path; argmax flips ~0).
"""
import sys

sys.path.insert(0, "/opt/trn_rl_repo")

import numpy as np
import ml_dtypes

import concourse.bass as bass  # noqa: F401
import concourse.tile as tile
from concourse import bacc, mybir
from concourse.bass_utils import run_bass_kernel_spmd

F32 = mybir.dt.float32
FP8 = mybir.dt.float8e4
BF16 = mybir.dt.bfloat16
ADD = mybir.AluOpType.add
SUB = mybir.AluOpType.subtract
MULT = mybir.AluOpType.mult
MIN = mybir.AluOpType.min
IDENT = mybir.ActivationFunctionType.Identity
EXP = mybir.ActivationFunctionType.Exp
AX = mybir.AxisListType.X

B, S, D = 8, 1024, 1024
H, DH = 16, 64
P = 128
NT = S // P
SCALE = 1.25e8  # 1e9 / 8

_CACHE = {}


def _bf16(a):
    return np.ascontiguousarray(a.astype(ml_dtypes.bfloat16))


def _build():
    nc = bacc.Bacc(None)

    xh_d = nc.dram_tensor("xh", [D, S], BF16, kind="ExternalInput")  # x[b].T hi
    xl_d = nc.dram_tensor("xl", [D, S], BF16, kind="ExternalInput")  # x[b].T lo
    m_d = nc.dram_tensor("m", [S, S], FP8, kind="ExternalInput")  # mask [i, j]
    wqkh_d = nc.dram_tensor("wqkh", [D, 2 * D], BF16, kind="ExternalInput")
    wqkl_d = nc.dram_tensor("wqkl", [D, 2 * D], BF16, kind="ExternalInput")
    wv_d = nc.dram_tensor("wv", [D, D], BF16, kind="ExternalInput")
    wp_d = nc.dram_tensor("wp", [D, D], BF16, kind="ExternalInput")
    bqk_lo_d = nc.dram_tensor("bqk_lo", [64, 16], F32, kind="ExternalInput")
    bqk_hi_d = nc.dram_tensor("bqk_hi", [64, 16], F32, kind="ExternalInput")
    bv_d = nc.dram_tensor("bv", [1, D], BF16, kind="ExternalInput")
    bp_d = nc.dram_tensor("bp", [1, D], BF16, kind="ExternalInput")
    y_d = nc.dram_tensor("y", [S, D], F32, kind="ExternalOutput")

    with tile.TileContext(nc) as tc:
        with (
            tc.tile_pool(name="res", bufs=1) as res,
            tc.tile_pool(name="qkres", bufs=1) as qkres,
            tc.tile_pool(name="odram", bufs=1, space="DRAM") as odram,
            tc.tile_pool(name="psA", bufs=2, space="PSUM") as psA,
        ):
            # ---- resident tiles ----
            qhl = qkres.tile([P, H, S], BF16, tag="qhl")  # [qh; ql] per q-head
            khh = qkres.tile([P, H, S], BF16, tag="khh")  # [kh; kh] per k-head
            klB = qkres.tile([P, H, S], BF16, tag="klB")  # [kl; kl] dup
            vmat = qkres.tile([P, NT, D], BF16, tag="vmat")  # [j_sub, j_tile, c]
            mposb = res.tile([P, NT, S], FP8, tag="mposb")  # [i_sub, i_tile, j]
            ones_row = res.tile([1, P], BF16, tag="ones")
            bqk_lo = res.tile([64, 16], F32, tag="bqk_lo")
            bqk_hi = res.tile([64, 16], F32, tag="bqk_hi")
            otm_d = odram.tile([P, NT, S], BF16, tag="otm_d")  # DRAM scratch

            nc.vector.memset(ones_row[:], 1.0)

            with tc.tile_pool(name="p12", bufs=1) as p12, \
                 tc.tile_pool(name="wstr", bufs=2) as wstr:
                xh = p12.tile([P, NT, S], BF16, tag="xh")  # [d_sub, d_tile, s]
                xl = p12.tile([P, NT, S], BF16, tag="xl")
                xh_r = xh_d.ap().rearrange("(t p) s -> p t s", p=P)
                # split startup DMAs so the first V matmuls start sooner
                nc.sync.dma_start(xh[:, 0:4, :], xh_r[:, 0:4, :])

                # ---------------- phase 1: V projection ----------------
                with tc.tile_pool(name="wvp", bufs=1) as wvp:
                    wv = wvp.tile([P, NT, D], BF16, tag="wv")
                    bv = wvp.tile([1, D], BF16, tag="bv")
                    wv_r = wv_d.ap().rearrange("(t p) c -> p t c", p=P)
                    nc.sync.dma_start(wv[:, 0:4, :], wv_r[:, 0:4, :])
                    nc.sync.dma_start(bv[:], bv_d[:])
                    nc.sync.dma_start(xh[:, 4:8, :], xh_r[:, 4:8, :])
                    nc.sync.dma_start(wv[:, 4:8, :], wv_r[:, 4:8, :])
                    nc.sync.dma_start(
                        xl[:], xl_d.ap().rearrange("(t p) s -> p t s", p=P))
                    nc.sync.dma_start(bqk_lo[:], bqk_lo_d[:])
                    nc.sync.dma_start(bqk_hi[:], bqk_hi_d[:])
                    for st in range(NT):
                        ssl = slice(st * P, (st + 1) * P)
                        for nh in range(2):
                            hsl = slice(nh * 512, (nh + 1) * 512)
                            ps = psA.tile([P, 512], F32, tag="ps")
                            for k in range(NT):
                                nc.tensor.matmul(
                                    ps[:], xh[:, k, ssl], wv[:, k, hsl],
                                    start=(k == 0), stop=False)
                            nc.tensor.matmul(
                                ps[:], ones_row[:], bv[:, hsl],
                                start=False, stop=True)
                            nc.scalar.copy(vmat[:, st, hsl], ps[:])
                    nc.sync.dma_start(
                        mposb[:], m_d.ap().rearrange("(t p) j -> p t j", p=P))

                # ------- phase 2+3: interleaved q/k proj + attention -------
                def proj_et(et):
                    is_q = et < 8
                    wh = wstr.tile([P, NT, P], BF16, tag="wh")
                    wl = wstr.tile([P, NT, P], BF16, tag="wl")
                    esl = slice(et * P, (et + 1) * P)
                    nc.sync.dma_start(
                        wh[:], wqkh_d[:, esl].rearrange("(t p) e -> p t e", p=P))
                    nc.sync.dma_start(
                        wl[:], wqkl_d[:, esl].rearrange("(t p) e -> p t e", p=P))
                    hA = 2 * (et % 8)
                    hB = hA + 1
                    for nh in range(2):
                        hsl = slice(nh * 512, (nh + 1) * 512)
                        ps = psA.tile([P, 512], F32, tag="ps")
                        first = True
                        for k in range(NT):
                            for mi, (wt, xt) in enumerate(
                                    ((wh, xh), (wl, xh), (wh, xl))):
                                nc.tensor.matmul(
                                    ps[:], wt[:, k, :], xt[:, k, hsl],
                                    start=first,
                                    stop=(k == NT - 1 and mi == 2))
                                first = False
                        if is_q:
                            nc.scalar.activation(
                                qhl[0:64, hA, hsl], ps[0:64], IDENT,
                                bias=bqk_lo[:, et:et + 1])
                            nc.scalar.activation(
                                qhl[0:64, hB, hsl], ps[64:128], IDENT,
                                bias=bqk_hi[:, et:et + 1])
                            nc.vector.scalar_tensor_tensor(
                                out=qhl[64:128, hA, hsl], in0=ps[0:64],
                                scalar=bqk_lo[:, et:et + 1],
                                in1=qhl[0:64, hA, hsl], op0=ADD, op1=SUB)
                            nc.vector.scalar_tensor_tensor(
                                out=qhl[64:128, hB, hsl], in0=ps[64:128],
                                scalar=bqk_hi[:, et:et + 1],
                                in1=qhl[0:64, hB, hsl], op0=ADD, op1=SUB)
                        else:
                            nc.scalar.activation(
                                khh[0:64, hA, hsl], ps[0:64], IDENT,
                                bias=bqk_lo[:, et:et + 1])
                            nc.scalar.activation(
                                khh[64:128, hA, hsl], ps[0:64], IDENT,
                                bias=bqk_lo[:, et:et + 1])
                            nc.scalar.activation(
                                khh[0:64, hB, hsl], ps[64:128], IDENT,
                                bias=bqk_hi[:, et:et + 1])
                            nc.scalar.activation(
                                khh[64:128, hB, hsl], ps[64:128], IDENT,
                                bias=bqk_hi[:, et:et + 1])
                            nc.vector.scalar_tensor_tensor(
                                out=klB[0:64, hA, hsl], in0=ps[0:64],
                                scalar=bqk_lo[:, et:et + 1],
                                in1=khh[0:64, hA, hsl], op0=ADD, op1=SUB)
                            nc.vector.scalar_tensor_tensor(
                                out=klB[0:64, hB, hsl], in0=ps[64:128],
                                scalar=bqk_hi[:, et:et + 1],
                                in1=khh[0:64, hB, hsl], op0=ADD, op1=SUB)
                            nc.scalar.copy(
                                klB[64:128, hA, hsl], klB[0:64, hA, hsl])
                            nc.scalar.copy(
                                klB[64:128, hB, hsl], klB[0:64, hB, hsl])

                with tc.tile_pool(name="ppool", bufs=2) as ppool, \
                     tc.tile_pool(name="ptpool", bufs=2) as ptpool, \
                     tc.tile_pool(name="ostg", bufs=2) as ostg, \
                     tc.tile_pool(name="ps_s", bufs=2, space="PSUM") as ps_s, \
                     tc.tile_pool(name="ps_o", bufs=2, space="PSUM") as ps_o:
                    ptbs = {}

                    def attn_front(h):
                        ptb = ptpool.tile([P, NT, S], BF16, tag="ptb")
                        ptbs[h] = ptb
                        for it in range(NT):
                            isl = slice(it * P, (it + 1) * P)
                            pss = ps_s.tile([P, S], F32, tag="pss")
                            for nh in range(2):
                                hsl = slice(nh * 512, (nh + 1) * 512)
                                nc.tensor.matmul(
                                    pss[:, hsl], qhl[:, h, isl],
                                    khh[:, h, hsl], start=True, stop=False)
                                nc.tensor.matmul(
                                    pss[:, hsl], qhl[:, h, isl],
                                    klB[:, h, hsl], start=False, stop=True)
                            # mask-mult to SBUF: frees the psum tile fast so
                            # the PE can run ahead (keeps the pstate ramped)
                            ut = ppool.tile([P, S], F32, tag="ut")
                            nc.vector.scalar_tensor_tensor(
                                out=ut[:], in0=pss[:], scalar=SCALE,
                                in1=mposb[:, it, :], op0=MULT, op1=MULT)
                            umin = ppool.tile([P, 1], F32, tag="umin")
                            nc.vector.tensor_reduce(
                                out=umin[:], in_=ut[:], axis=AX, op=MIN)
                            pt = ppool.tile([P, S], BF16, tag="pt")
                            nc.scalar.activation(
                                out=pt[:], in_=ut[:], func=EXP,
                                bias=umin[:], scale=-1.0)
                            nc.sync.dma_start_transpose(ptb[:, :, isl], pt[:])

                    def attn_pv(h):
                        ptb = ptbs.pop(h)
                        csl = slice((h // 2) * P + (h % 2) * 64,
                                    (h // 2) * P + (h % 2) * 64 + 64)
                        obase = (h % 2) * 64
                        for nh in range(2):
                            hsl = slice(nh * 512, (nh + 1) * 512)
                            pso = ps_o.tile([64, 512], F32, tag="pso")
                            for jt in range(NT):
                                nc.tensor.matmul(
                                    pso[:], vmat[:, jt, csl], ptb[:, jt, hsl],
                                    start=(jt == 0), stop=(jt == NT - 1))
                            og = ostg.tile([64, 512], BF16, tag="og")
                            nc.scalar.copy(og[:], pso[:])
                            nc.sync.dma_start(
                                otm_d[obase:obase + 64, h // 2, hsl], og[:])

                    # attention lags projection by one head-pair so the
                    # proj-subs (DVE) and copies (Act) for the next heads are
                    # issued ahead of the current heads' stt/reduce/exp
                    # backlog on those same in-order engines.
                    proj_et(0)
                    proj_et(8)
                    for i in range(1, 8):
                        proj_et(i)
                        proj_et(8 + i)
                        attn_front(2 * i - 2)
                        if i > 1:
                            attn_pv(2 * i - 3)
                        attn_front(2 * i - 1)
                        attn_pv(2 * i - 2)
                    attn_front(14)
                    attn_pv(13)
                    attn_front(15)
                    attn_pv(14)
                    attn_pv(15)

            # ---------------- phase 4: output projection ----------------
            with tc.tile_pool(name="late", bufs=1) as late, \
                 tc.tile_pool(name="ypool", bufs=2) as ypool:
                wpt = late.tile([P, NT, D], BF16, tag="wp")
                bp = late.tile([1, D], BF16, tag="bp")
                otm = late.tile([P, NT, S], BF16, tag="otm")
                nc.sync.dma_start(
                    wpt[:], wp_d.ap().rearrange("(t p) d -> p t d", p=P))
                nc.sync.dma_start(bp[:], bp_d[:])
                # per-ot reload: each c-tile only waits for its two heads' PV
                for ot in range(NT):
                    nc.sync.dma_start(otm[:, ot, :], otm_d[:, ot, :])
                for st in range(NT):
                    ssl = slice(st * P, (st + 1) * P)
                    yt = ypool.tile([P, D], F32, tag="yt")
                    for nh in range(2):
                        hsl = slice(nh * 512, (nh + 1) * 512)
                        ps = psA.tile([P, 512], F32, tag="ps")
                        for ot in range(NT):
                            nc.tensor.matmul(
                                ps[:], otm[:, ot, ssl], wpt[:, ot, hsl],
                                start=(ot == 0), stop=False)
                        nc.tensor.matmul(
                            ps[:], ones_row[:], bp[:, hsl],
                            start=False, stop=True)
                        nc.scalar.copy(yt[:, hsl], ps[:])
                    nc.sync.dma_start(y_d[st * P:(st + 1) * P, :], yt[:])

    nc.compile()
    return nc


def _prep_inputs(x, mask, W_attn, b_attn, W_proj, b_proj):
    x = np.asarray(x, np.float32)
    mask = np.asarray(mask, np.float32)
    W_attn = np.asarray(W_attn, np.float32)
    b_attn = np.asarray(b_attn, np.float32).reshape(-1)
    W_proj = np.asarray(W_proj, np.float32)
    b_proj = np.asarray(b_proj, np.float32).reshape(-1)

    wqk = W_attn[:, : 2 * D]
    wqkh = _bf16(wqk)
    wqkl = _bf16(wqk - wqkh.astype(np.float32))
    wv = _bf16(W_attn[:, 2 * D:])
    wp = _bf16(W_proj)

    bqk = b_attn[: 2 * D].reshape(16, 128)  # [et, p]
    bqk_lo = np.ascontiguousarray(bqk[:, 0:64].T, np.float32)  # [64, 16]
    bqk_hi = np.ascontiguousarray(bqk[:, 64:128].T, np.float32)
    bv = _bf16(b_attn[2 * D:].reshape(1, D))
    bp = _bf16(b_proj.reshape(1, D))

    shared = dict(wqkh=wqkh, wqkl=wqkl, wv=wv, wp=wp,
                  bqk_lo=bqk_lo, bqk_hi=bqk_hi, bv=bv, bp=bp)
    in_maps = []
    for b in range(B):
        xT = np.ascontiguousarray(x[b].T)
        xh = _bf16(xT)
        xli = _bf16(xT - xh.astype(np.float32))
        in_maps.append(dict(
            xh=xh, xl=xli,
            m=np.ascontiguousarray(mask[b, 0].astype(ml_dtypes.float8_e4m3)),
            **shared))
    return in_maps


def kernel(x, mask, W_attn, b_attn, W_proj, b_proj, _trace=False, _trace_kwargs=None):
    if "nc" not in _CACHE:
        _CACHE["nc"] = _build()
    nc = _CACHE["nc"]
    in_maps = _prep_inputs(x, mask, W_attn, b_attn, W_proj, b_proj)
    kw = {}
    if _trace:
        kw = dict(trace=True, **(_trace_kwargs or {}))
    res = run_bass_kernel_spmd(nc, in_maps, core_ids=list(range(B)), **kw)
    out = np.stack([res.results[b]["y"] for b in range(B)], axis=0)
    if _trace:
        _CACHE["last_results"] = res
    return out


# revision 25
# speedup vs baseline: 1.2083x; 1.0046x over previous
"""Multi-head attention (degenerate multiplicative-mask softmax) on 8 TRN2 cores.

Sharding: pure data-parallel over batch (B=8 -> 1 batch element per core).
No collectives. Each core computes its batch's full attention + output proj.

v3 design (v1 757us -> v2 623us -> v3):
  - Scores: 3-pass bf16 hi/lo packed into 2 matmuls: K=128 [qh;ql]x[kh;kh]
    + K=64 qh x kl. Same numerics as v1, 2/3 the PE time.
  - P^T built by DMA xbar transposes (dma_start_transpose, 3D out) on the
    otherwise-idle DMA engines - no PE transposes, no psum->sbuf copies.
  - FULL proj/attention interleave: after each (q-et, k-et) projection pair,
    the two ready heads' attention is emitted, so vector/scalar/DMA attention
    work overlaps the projection matmul stream and TensorE becomes the only
    wall. SBUF fits because: masked scores live in PSUM (no ut tile), the
    attention output otm round-trips through a DRAM tile (saves 16K/part),
    mask is fp8, and phase-4 weights load into a late-scoped pool.
  - PV runs one head behind the scores pipeline (ptb double-buffered), so
    the in-order PE never stalls on the xbar transposes.
  - Biases folded: q/k exact-f32 via activation(Identity, bias) on the
    psum->sbuf copy; v/proj via single K=1 ones-matmul.

Precision: identical hi/lo 3-pass scheme as v1 (rel err 0.0054, all from the
single-bf16 V path; argmax flips ~0).
"""
import sys

sys.path.insert(0, "/opt/trn_rl_repo")

import numpy as np
import ml_dtypes

import concourse.bass as bass  # noqa: F401
import concourse.tile as tile
from concourse import bacc, mybir
from concourse.bass_utils import run_bass_kernel_spmd

F32 = mybir.dt.float32
FP8 = mybir.dt.float8e4
BF16 = mybir.dt.bfloat16
ADD = mybir.AluOpType.add
SUB = mybir.AluOpType.subtract
MULT = mybir.AluOpType.mult
MIN = mybir.AluOpType.min
IDENT = mybir.ActivationFunctionType.Identity
EXP = mybir.ActivationFunctionType.Exp
AX = mybir.AxisListType.X

B, S, D = 8, 1024, 1024
H, DH = 16, 64
P = 128
NT = S // P
SCALE = 1.25e8  # 1e9 / 8

_CACHE = {}


def _bf16(a):
    return np.ascontiguousarray(a.astype(ml_dtypes.bfloat16))


def _build():
    nc = bacc.Bacc(None)

    xh_d = nc.dram_tensor("xh", [D, S], BF16, kind="ExternalInput")  # x[b].T hi
    xl_d = nc.dram_tensor("xl", [D, S], BF16, kind="ExternalInput")  # x[b].T lo
    m_d = nc.dram_tensor("m", [S, S], FP8, kind="ExternalInput")  # mask [i, j]
    wqkh_d = nc.dram_tensor("wqkh", [D, 2 * D], BF16, kind="ExternalInput")
    wqkl_d = nc.dram_tensor("wqkl", [D, 2 * D], BF16, kind="ExternalInput")
    wv_d = nc.dram_tensor("wv", [D, D], BF16, kind="ExternalInput")
    wp_d = nc.dram_tensor("wp", [D, D], BF16, kind="ExternalInput")
    bqk_lo_d = nc.dram_tensor("bqk_lo", [64, 16], F32, kind="ExternalInput")
    bqk_hi_d = nc.dram_tensor("bqk_hi", [64, 16], F32, kind="ExternalInput")
    bv_d = nc.dram_tensor("bv", [1, D], BF16, kind="ExternalInput")
    bp_d = nc.dram_tensor("bp", [1, D], BF16, kind="ExternalInput")
    y_d = nc.dram_tensor("y", [S, D], F32, kind="ExternalOutput")

    with tile.TileContext(nc) as tc:
        with (
            tc.tile_pool(name="res", bufs=1) as res,
            tc.tile_pool(name="qkres", bufs=1) as qkres,
            tc.tile_pool(name="odram", bufs=1, space="DRAM") as odram,
            tc.tile_pool(name="psA", bufs=2, space="PSUM") as psA,
        ):
            # ---- resident tiles ----
            qhl = qkres.tile([P, H, S], BF16, tag="qhl")  # [qh; ql] per q-head
            khh = qkres.tile([P, H, S], BF16, tag="khh")  # [kh; kh] per k-head
            klB = qkres.tile([P, H, S], BF16, tag="klB")  # [kl; kl] dup
            vmat = qkres.tile([P, NT, D], BF16, tag="vmat")  # [j_sub, j_tile, c]
            mposb = res.tile([P, NT, S], FP8, tag="mposb")  # [i_sub, i_tile, j]
            ones_row = res.tile([1, P], BF16, tag="ones")
            bqk_lo = res.tile([64, 16], F32, tag="bqk_lo")
            bqk_hi = res.tile([64, 16], F32, tag="bqk_hi")
            otm_d = odram.tile([P, NT, S], BF16, tag="otm_d")  # DRAM scratch

            nc.vector.memset(ones_row[:], 1.0)

            with tc.tile_pool(name="p12", bufs=1) as p12, \
                 tc.tile_pool(name="wstr", bufs=2) as wstr:
                xh = p12.tile([P, NT, S], BF16, tag="xh")  # [d_sub, d_tile, s]
                xl = p12.tile([P, NT, S], BF16, tag="xl")
                xh_r = xh_d.ap().rearrange("(t p) s -> p t s", p=P)
                # split startup DMAs so the first V matmuls start sooner
                nc.sync.dma_start(xh[:, 0:4, :], xh_r[:, 0:4, :])

                # ---------------- phase 1: V projection ----------------
                with tc.tile_pool(name="wvp", bufs=1) as wvp:
                    wv = wvp.tile([P, NT, D], BF16, tag="wv")
                    bv = wvp.tile([1, D], BF16, tag="bv")
                    wv_r = wv_d.ap().rearrange("(t p) c -> p t c", p=P)
                    nc.sync.dma_start(bv[:], bv_d[:])
                    nc.sync.dma_start(wv[:, 0:4, :], wv_r[:, 0:4, :])
                    nc.sync.dma_start(xh[:, 4:8, :], xh_r[:, 4:8, :])
                    nc.sync.dma_start(wv[:, 4:8, :], wv_r[:, 4:8, :])
                    nc.sync.dma_start(
                        xl[:], xl_d.ap().rearrange("(t p) s -> p t s", p=P))
                    nc.sync.dma_start(bqk_lo[:], bqk_lo_d[:])
                    nc.sync.dma_start(bqk_hi[:], bqk_hi_d[:])
                    for st in range(NT):
                        ssl = slice(st * P, (st + 1) * P)
                        for nh in range(2):
                            hsl = slice(nh * 512, (nh + 1) * 512)
                            ps = psA.tile([P, 512], F32, tag="ps")
                            for k in range(NT):
                                nc.tensor.matmul(
                                    ps[:], xh[:, k, ssl], wv[:, k, hsl],
                                    start=(k == 0), stop=False)
                            nc.tensor.matmul(
                                ps[:], ones_row[:], bv[:, hsl],
                                start=False, stop=True)
                            nc.scalar.copy(vmat[:, st, hsl], ps[:])
                    nc.sync.dma_start(
                        mposb[:], m_d.ap().rearrange("(t p) j -> p t j", p=P))

                # ------- phase 2+3: interleaved q/k proj + attention -------
                def proj_et(et):
                    is_q = et < 8
                    wh = wstr.tile([P, NT, P], BF16, tag="wh")
                    wl = wstr.tile([P, NT, P], BF16, tag="wl")
                    esl = slice(et * P, (et + 1) * P)
                    nc.sync.dma_start(
                        wh[:], wqkh_d[:, esl].rearrange("(t p) e -> p t e", p=P))
                    nc.sync.dma_start(
                        wl[:], wqkl_d[:, esl].rearrange("(t p) e -> p t e", p=P))
                    hA = 2 * (et % 8)
                    hB = hA + 1
                    for nh in range(2):
                        hsl = slice(nh * 512, (nh + 1) * 512)
                        ps = psA.tile([P, 512], F32, tag="ps")
                        first = True
                        for k in range(NT):
                            for mi, (wt, xt) in enumerate(
                                    ((wh, xh), (wl, xh), (wh, xl))):
                                nc.tensor.matmul(
                                    ps[:], wt[:, k, :], xt[:, k, hsl],
                                    start=first,
                                    stop=(k == NT - 1 and mi == 2))
                                first = False
                        if is_q:
                            nc.scalar.activation(
                                qhl[0:64, hA, hsl], ps[0:64], IDENT,
                                bias=bqk_lo[:, et:et + 1])
                            nc.scalar.activation(
                                qhl[0:64, hB, hsl], ps[64:128], IDENT,
                                bias=bqk_hi[:, et:et + 1])
                            nc.vector.scalar_tensor_tensor(
                                out=qhl[64:128, hA, hsl], in0=ps[0:64],
                                scalar=bqk_lo[:, et:et + 1],
                                in1=qhl[0:64, hA, hsl], op0=ADD, op1=SUB)
                            nc.vector.scalar_tensor_tensor(
                                out=qhl[64:128, hB, hsl], in0=ps[64:128],
                                scalar=bqk_hi[:, et:et + 1],
                                in1=qhl[0:64, hB, hsl], op0=ADD, op1=SUB)
                        else:
                            nc.scalar.activation(
                                khh[0:64, hA, hsl], ps[0:64], IDENT,
                                bias=bqk_lo[:, et:et + 1])
                            nc.scalar.activation(
                                khh[64:128, hA, hsl], ps[0:64], IDENT,
                                bias=bqk_lo[:, et:et + 1])
                            nc.scalar.activation(
                                khh[0:64, hB, hsl], ps[64:128], IDENT,
                                bias=bqk_hi[:, et:et + 1])
                            nc.scalar.activation(
                                khh[64:128, hB, hsl], ps[64:128], IDENT,
                                bias=bqk_hi[:, et:et + 1])
                            nc.vector.scalar_tensor_tensor(
                                out=klB[0:64, hA, hsl], in0=ps[0:64],
                                scalar=bqk_lo[:, et:et + 1],
                                in1=khh[0:64, hA, hsl], op0=ADD, op1=SUB)
                            nc.vector.scalar_tensor_tensor(
                                out=klB[0:64, hB, hsl], in0=ps[64:128],
                                scalar=bqk_hi[:, et:et + 1],
                                in1=khh[0:64, hB, hsl], op0=ADD, op1=SUB)
                            nc.scalar.copy(
                                klB[64:128, hA, hsl], klB[0:64, hA, hsl])
                            nc.scalar.copy(
                                klB[64:128, hB, hsl], klB[0:64, hB, hsl])

                with tc.tile_pool(name="ppool", bufs=2) as ppool, \
                     tc.tile_pool(name="ptpool", bufs=2) as ptpool, \
                     tc.tile_pool(name="ostg", bufs=2) as ostg, \
                     tc.tile_pool(name="ps_s", bufs=2, space="PSUM") as ps_s, \
                     tc.tile_pool(name="ps_o", bufs=2, space="PSUM") as ps_o:
                    ptbs = {}

                    def attn_front(h):
                        ptb = ptpool.tile([P, NT, S], BF16, tag="ptb")
                        ptbs[h] = ptb
                        for it in range(NT):
                            isl = slice(it * P, (it + 1) * P)
                            pss = ps_s.tile([P, S], F32, tag="pss")
                            for nh in range(2):
                                hsl = slice(nh * 512, (nh + 1) * 512)
                                nc.tensor.matmul(
                                    pss[:, hsl], qhl[:, h, isl],
                                    khh[:, h, hsl], start=True, stop=False)
                                nc.tensor.matmul(
                                    pss[:, hsl], qhl[:, h, isl],
                                    klB[:, h, hsl], start=False, stop=True)
                            # mask-mult to SBUF: frees the psum tile fast so
                            # the PE can run ahead (keeps the pstate ramped)
                            ut = ppool.tile([P, S], F32, tag="ut")
                            nc.vector.scalar_tensor_tensor(
                                out=ut[:], in0=pss[:], scalar=SCALE,
                                in1=mposb[:, it, :], op0=MULT, op1=MULT)
                            umin = ppool.tile([P, 1], F32, tag="umin")
                            nc.vector.tensor_reduce(
                                out=umin[:], in_=ut[:], axis=AX, op=MIN)
                            pt = ppool.tile([P, S], BF16, tag="pt")
                            nc.scalar.activation(
                                out=pt[:], in_=ut[:], func=EXP,
                                bias=umin[:], scale=-1.0)
                            nc.sync.dma_start_transpose(ptb[:, :, isl], pt[:])

                    def attn_pv(h):
                        ptb = ptbs.pop(h)
                        csl = slice((h // 2) * P + (h % 2) * 64,
                                    (h // 2) * P + (h % 2) * 64 + 64)
                        obase = (h % 2) * 64
                        for nh in range(2):
                            hsl = slice(nh * 512, (nh + 1) * 512)
                            pso = ps_o.tile([64, 512], F32, tag="pso")
                            for jt in range(NT):
                                nc.tensor.matmul(
                                    pso[:], vmat[:, jt, csl], ptb[:, jt, hsl],
                                    start=(jt == 0), stop=(jt == NT - 1))
                            og = ostg.tile([64, 512], BF16, tag="og")
                            nc.scalar.copy(og[:], pso[:])
                            nc.sync.dma_start(
                                otm_d[obase:obase + 64, h // 2, hsl], og[:])

                    # attention lags projection by one head-pair so the
                    # proj-subs (DVE) and copies (Act) for the next heads are
                    # issued ahead of the current heads' stt/reduce/exp
                    # backlog on those same in-order engines.
                    proj_et(0)
                    proj_et(8)
                    for i in range(1, 8):
                        proj_et(i)
                        proj_et(8 + i)
                        attn_front(2 * i - 2)
                        if i > 1:
                            attn_pv(2 * i - 3)
                        attn_front(2 * i - 1)
                        attn_pv(2 * i - 2)
                    attn_front(14)
                    attn_pv(13)
                    attn_front(15)
                    attn_pv(14)
                    attn_pv(15)

            # ---------------- phase 4: output projection ----------------
            with tc.tile_pool(name="late", bufs=1) as late, \
                 tc.tile_pool(name="ypool", bufs=2) as ypool:
                wpt = late.tile([P, NT, D], BF16, tag="wp")
                bp = late.tile([1, D], BF16, tag="bp")
                otm = late.tile([P, NT, S], BF16, tag="otm")
                nc.scalar.dma_start(
                    wpt[:], wp_d.ap().rearrange("(t p) d -> p t d", p=P))
                nc.scalar.dma_start(bp[:], bp_d[:])
                # per-ot reload: each c-tile only waits for its two heads' PV
                # (same sync queue as the og writes: FIFO-safe)
                for ot in range(NT):
                    nc.sync.dma_start(otm[:, ot, :], otm_d[:, ot, :])
                for st in range(NT):
                    ssl = slice(st * P, (st + 1) * P)
                    yt = ypool.tile([P, D], F32, tag="yt")
                    for nh in range(2):
                        hsl = slice(nh * 512, (nh + 1) * 512)
                        ps = psA.tile([P, 512], F32, tag="ps")
                        for ot in range(NT):
                            nc.tensor.matmul(
                                ps[:], otm[:, ot, ssl], wpt[:, ot, hsl],
                                start=(ot == 0), stop=False)
                        nc.tensor.matmul(
                            ps[:], ones_row[:], bp[:, hsl],
                            start=False, stop=True)
                        nc.scalar.copy(yt[:, hsl], ps[:])
                    nc.sync.dma_start(y_d[st * P:(st + 1) * P, :], yt[:])

    nc.compile()
    return nc


def _prep_inputs(x, mask, W_attn, b_attn, W_proj, b_proj):
    x = np.asarray(x, np.float32)
    mask = np.asarray(mask, np.float32)
    W_attn = np.asarray(W_attn, np.float32)
    b_attn = np.asarray(b_attn, np.float32).reshape(-1)
    W_proj = np.asarray(W_proj, np.float32)
    b_proj = np.asarray(b_proj, np.float32).reshape(-1)

    wqk = W_attn[:, : 2 * D]
    wqkh = _bf16(wqk)
    wqkl = _bf16(wqk - wqkh.astype(np.float32))
    wv = _bf16(W_attn[:, 2 * D:])
    wp = _bf16(W_proj)

    bqk = b_attn[: 2 * D].reshape(16, 128)  # [et, p]
    bqk_lo = np.ascontiguousarray(bqk[:, 0:64].T, np.float32)  # [64, 16]
    bqk_hi = np.ascontiguousarray(bqk[:, 64:128].T, np.float32)
    bv = _bf16(b_attn[2 * D:].reshape(1, D))
    bp = _bf16(b_proj.reshape(1, D))

    shared = dict(wqkh=wqkh, wqkl=wqkl, wv=wv, wp=wp,
                  bqk_lo=bqk_lo, bqk_hi=bqk_hi, bv=bv, bp=bp)
    in_maps = []
    for b in range(B):
        xT = np.ascontiguousarray(x[b].T)
        xh = _bf16(xT)
        xli = _bf16(xT - xh.astype(np.float32))
        in_maps.append(dict(
            xh=xh, xl=xli,
            m=np.ascontiguousarray(mask[b, 0].astype(ml_dtypes.float8_e4m3)),
            **shared))
    return in_maps


def kernel(x, mask, W_attn, b_attn, W_proj, b_proj, _trace=False, _trace_kwargs=None):
    if "nc" not in _CACHE:
        _CACHE["nc"] = _build()
    nc = _CACHE["nc"]
    in_maps = _prep_inputs(x, mask, W_attn, b_attn, W_proj, b_proj)
    kw = {}
    if _trace:
        kw = dict(trace=True, **(_trace_kwargs or {}))
    res = run_bass_kernel_spmd(nc, in_maps, core_ids=list(range(B)), **kw)
    out = np.stack([res.results[b]["y"] for b in range(B)], axis=0)
    if _trace:
        _CACHE["last_results"] = res
    return out


# revision 26
# speedup vs baseline: 1.2377x; 1.0243x over previous
"""Multi-head attention (degenerate multiplicative-mask softmax) on 8 TRN2 cores.

Sharding: pure data-parallel over batch (B=8 -> 1 batch element per core).
No collectives. Each core computes its batch's full attention + output proj.

v3 design (v1 757us -> v2 623us -> v3):
  - Scores: 3-pass bf16 hi/lo packed into 2 matmuls: K=128 [qh;ql]x[kh;kh]
    + K=64 qh x kl. Same numerics as v1, 2/3 the PE time.
  - P^T built by DMA xbar transposes (dma_start_transpose, 3D out) on the
    otherwise-idle DMA engines - no PE transposes, no psum->sbuf copies.
  - FULL proj/attention interleave: after each (q-et, k-et) projection pair,
    the two ready heads' attention is emitted, so vector/scalar/DMA attention
    work overlaps the projection matmul stream and TensorE becomes the only
    wall. SBUF fits because: masked scores live in PSUM (no ut tile), the
    attention output otm round-trips through a DRAM tile (saves 16K/part),
    mask is fp8, and phase-4 weights load into a late-scoped pool.
  - PV runs one head behind the scores pipeline (ptb double-buffered), so
    the in-order PE never stalls on the xbar transposes.
  - Biases folded: q/k exact-f32 via activation(Identity, bias) on the
    psum->sbuf copy; v/proj via single K=1 ones-matmul.

Precision: identical hi/lo 3-pass scheme as v1 (rel err 0.0054, all from the
single-bf16 V path; argmax flips ~0).
"""
import sys

sys.path.insert(0, "/opt/trn_rl_repo")

import numpy as np
import ml_dtypes

import concourse.bass as bass  # noqa: F401
import concourse.tile as tile
from concourse import bacc, mybir
from concourse.bass_utils import run_bass_kernel_spmd

F32 = mybir.dt.float32
FP8 = mybir.dt.float8e4
BF16 = mybir.dt.bfloat16
ADD = mybir.AluOpType.add
SUB = mybir.AluOpType.subtract
MULT = mybir.AluOpType.mult
MIN = mybir.AluOpType.min
IDENT = mybir.ActivationFunctionType.Identity
EXP = mybir.ActivationFunctionType.Exp
AX = mybir.AxisListType.X

B, S, D = 8, 1024, 1024
H, DH = 16, 64
P = 128
NT = S // P
SCALE = 1.25e8  # 1e9 / 8

_CACHE = {}


def _bf16(a):
    return np.ascontiguousarray(a.astype(ml_dtypes.bfloat16))


def _build():
    nc = bacc.Bacc(None)

    xh_d = nc.dram_tensor("xh", [D, S], BF16, kind="ExternalInput")  # x[b].T hi
    xl_d = nc.dram_tensor("xl", [D, S], BF16, kind="ExternalInput")  # x[b].T lo
    m_d = nc.dram_tensor("m", [S, S], FP8, kind="ExternalInput")  # mask [i, j]
    wqkh_d = nc.dram_tensor("wqkh", [D, 2 * D], BF16, kind="ExternalInput")
    wqkl_d = nc.dram_tensor("wqkl", [D, 2 * D], BF16, kind="ExternalInput")
    wv_d = nc.dram_tensor("wv", [D, D], BF16, kind="ExternalInput")
    wp_d = nc.dram_tensor("wp", [D, D], BF16, kind="ExternalInput")
    bqk_lo_d = nc.dram_tensor("bqk_lo", [64, 16], F32, kind="ExternalInput")
    bqk_hi_d = nc.dram_tensor("bqk_hi", [64, 16], F32, kind="ExternalInput")
    bv_d = nc.dram_tensor("bv", [1, D], BF16, kind="ExternalInput")
    bp_d = nc.dram_tensor("bp", [1, D], BF16, kind="ExternalInput")
    y_d = nc.dram_tensor("y", [S, D], F32, kind="ExternalOutput")

    with tile.TileContext(nc) as tc:
        with (
            tc.tile_pool(name="res", bufs=1) as res,
            tc.tile_pool(name="qkres", bufs=1) as qkres,
            tc.tile_pool(name="odram", bufs=1, space="DRAM") as odram,
            tc.tile_pool(name="psA", bufs=2, space="PSUM") as psA,
        ):
            # ---- resident tiles ----
            qhl = qkres.tile([P, H, S], BF16, tag="qhl")  # [qh; ql] per q-head
            khh = qkres.tile([P, H, S], BF16, tag="khh")  # [kh; kh] per k-head
            klB = qkres.tile([P, H, S], BF16, tag="klB")  # [kl; kl] dup
            vmat = qkres.tile([P, NT, D], BF16, tag="vmat")  # [j_sub, j_tile, c]
            mposb = res.tile([P, NT, S], FP8, tag="mposb")  # [i_sub, i_tile, j]
            ones_row = res.tile([1, P], BF16, tag="ones")
            bqk_lo = res.tile([64, 16], F32, tag="bqk_lo")
            bqk_hi = res.tile([64, 16], F32, tag="bqk_hi")
            otm_d = odram.tile([P, NT, S], BF16, tag="otm_d")  # DRAM scratch

            nc.vector.memset(ones_row[:], 1.0)

            with tc.tile_pool(name="p12", bufs=1) as p12, \
                 tc.tile_pool(name="wstr", bufs=2) as wstr:
                xh = p12.tile([P, NT, S], BF16, tag="xh")  # [d_sub, d_tile, s]
                xl = p12.tile([P, NT, S], BF16, tag="xl")
                xh_r = xh_d.ap().rearrange("(t p) s -> p t s", p=P)
                # split startup DMAs so the first V matmuls start sooner
                nc.sync.dma_start(xh[:, 0:4, :], xh_r[:, 0:4, :])

                # ---------------- phase 1: V projection ----------------
                with tc.tile_pool(name="wvp", bufs=1) as wvp:
                    wv = wvp.tile([P, NT, D], BF16, tag="wv")
                    bv = wvp.tile([1, D], BF16, tag="bv")
                    wv_r = wv_d.ap().rearrange("(t p) c -> p t c", p=P)
                    bvb = wvp.tile([P, D], BF16, tag="bvb")
                    nc.sync.dma_start(bv[:], bv_d[:])
                    nc.sync.dma_start(wv[:, 0:4, :], wv_r[:, 0:4, :])
                    nc.sync.dma_start(xh[:, 4:8, :], xh_r[:, 4:8, :])
                    nc.sync.dma_start(wv[:, 4:8, :], wv_r[:, 4:8, :])
                    nc.sync.dma_start(
                        xl[:], xl_d.ap().rearrange("(t p) s -> p t s", p=P))
                    nc.sync.dma_start(bqk_lo[:], bqk_lo_d[:])
                    nc.sync.dma_start(bqk_hi[:], bqk_hi_d[:])
                    bps0 = psA.tile([P, 512], F32, tag="ps")
                    nc.tensor.matmul(bps0[:], ones_row[:], bv[:, 0:512],
                                     start=True, stop=True)
                    nc.scalar.copy(bvb[:, 0:512], bps0[:])
                    bps1 = psA.tile([P, 512], F32, tag="ps")
                    nc.tensor.matmul(bps1[:], ones_row[:], bv[:, 512:1024],
                                     start=True, stop=True)
                    nc.scalar.copy(bvb[:, 512:1024], bps1[:])
                    for st in range(NT):
                        ssl = slice(st * P, (st + 1) * P)
                        for nh in range(2):
                            hsl = slice(nh * 512, (nh + 1) * 512)
                            ps = psA.tile([P, 512], F32, tag="ps")
                            for k in range(NT):
                                nc.tensor.matmul(
                                    ps[:], xh[:, k, ssl], wv[:, k, hsl],
                                    start=(k == 0), stop=(k == NT - 1))
                            nc.vector.scalar_tensor_tensor(
                                out=vmat[:, st, hsl], in0=ps[:], scalar=0.0,
                                in1=bvb[:, hsl], op0=ADD, op1=ADD)
                    nc.sync.dma_start(
                        mposb[:], m_d.ap().rearrange("(t p) j -> p t j", p=P))

                # ------- phase 2+3: interleaved q/k proj + attention -------
                def proj_et(et):
                    is_q = et < 8
                    wh = wstr.tile([P, NT, P], BF16, tag="wh")
                    wl = wstr.tile([P, NT, P], BF16, tag="wl")
                    esl = slice(et * P, (et + 1) * P)
                    nc.sync.dma_start(
                        wh[:], wqkh_d[:, esl].rearrange("(t p) e -> p t e", p=P))
                    nc.sync.dma_start(
                        wl[:], wqkl_d[:, esl].rearrange("(t p) e -> p t e", p=P))
                    hA = 2 * (et % 8)
                    hB = hA + 1
                    for nh in range(2):
                        hsl = slice(nh * 512, (nh + 1) * 512)
                        ps = psA.tile([P, 512], F32, tag="ps")
                        first = True
                        for k in range(NT):
                            for mi, (wt, xt) in enumerate(
                                    ((wh, xh), (wl, xh), (wh, xl))):
                                nc.tensor.matmul(
                                    ps[:], wt[:, k, :], xt[:, k, hsl],
                                    start=first,
                                    stop=(k == NT - 1 and mi == 2))
                                first = False
                        if is_q:
                            nc.scalar.activation(
                                qhl[0:64, hA, hsl], ps[0:64], IDENT,
                                bias=bqk_lo[:, et:et + 1])
                            nc.scalar.activation(
                                qhl[0:64, hB, hsl], ps[64:128], IDENT,
                                bias=bqk_hi[:, et:et + 1])
                            nc.vector.scalar_tensor_tensor(
                                out=qhl[64:128, hA, hsl], in0=ps[0:64],
                                scalar=bqk_lo[:, et:et + 1],
                                in1=qhl[0:64, hA, hsl], op0=ADD, op1=SUB)
                            nc.vector.scalar_tensor_tensor(
                                out=qhl[64:128, hB, hsl], in0=ps[64:128],
                                scalar=bqk_hi[:, et:et + 1],
                                in1=qhl[0:64, hB, hsl], op0=ADD, op1=SUB)
                        else:
                            nc.scalar.activation(
                                khh[0:64, hA, hsl], ps[0:64], IDENT,
                                bias=bqk_lo[:, et:et + 1])
                            nc.scalar.activation(
                                khh[64:128, hA, hsl], ps[0:64], IDENT,
                                bias=bqk_lo[:, et:et + 1])
                            nc.scalar.activation(
                                khh[0:64, hB, hsl], ps[64:128], IDENT,
                                bias=bqk_hi[:, et:et + 1])
                            nc.scalar.activation(
                                khh[64:128, hB, hsl], ps[64:128], IDENT,
                                bias=bqk_hi[:, et:et + 1])
                            nc.vector.scalar_tensor_tensor(
                                out=klB[0:64, hA, hsl], in0=ps[0:64],
                                scalar=bqk_lo[:, et:et + 1],
                                in1=khh[0:64, hA, hsl], op0=ADD, op1=SUB)
                            nc.vector.scalar_tensor_tensor(
                                out=klB[0:64, hB, hsl], in0=ps[64:128],
                                scalar=bqk_hi[:, et:et + 1],
                                in1=khh[0:64, hB, hsl], op0=ADD, op1=SUB)
                            nc.scalar.copy(
                                klB[64:128, hA, hsl], klB[0:64, hA, hsl])
                            nc.scalar.copy(
                                klB[64:128, hB, hsl], klB[0:64, hB, hsl])

                with tc.tile_pool(name="ppool", bufs=2) as ppool, \
                     tc.tile_pool(name="ptpool", bufs=2) as ptpool, \
                     tc.tile_pool(name="ostg", bufs=2) as ostg, \
                     tc.tile_pool(name="ps_s", bufs=2, space="PSUM") as ps_s, \
                     tc.tile_pool(name="ps_o", bufs=2, space="PSUM") as ps_o:
                    ptbs = {}

                    def attn_front(h):
                        ptbA = ptpool.tile([P, NT, 512], BF16, tag="ptbA")
                        ptbB = ptpool.tile([P, NT, 512], BF16, tag="ptbB")
                        ptbs[h] = (ptbA, ptbB)
                        for it in range(NT):
                            isl = slice(it * P, (it + 1) * P)
                            pss = ps_s.tile([P, S], F32, tag="pss")
                            for nh in range(2):
                                hsl = slice(nh * 512, (nh + 1) * 512)
                                nc.tensor.matmul(
                                    pss[:, hsl], qhl[:, h, isl],
                                    khh[:, h, hsl], start=True, stop=False)
                                nc.tensor.matmul(
                                    pss[:, hsl], qhl[:, h, isl],
                                    klB[:, h, hsl], start=False, stop=True)
                            # mask-mult to SBUF: frees the psum tile fast so
                            # the PE can run ahead (keeps the pstate ramped)
                            ut = ppool.tile([P, S], F32, tag="ut")
                            nc.vector.scalar_tensor_tensor(
                                out=ut[:], in0=pss[:], scalar=SCALE,
                                in1=mposb[:, it, :], op0=MULT, op1=MULT)
                            umin = ppool.tile([P, 1], F32, tag="umin")
                            nc.vector.tensor_reduce(
                                out=umin[:], in_=ut[:], axis=AX, op=MIN)
                            pt = ppool.tile([P, S], BF16, tag="pt")
                            nc.scalar.activation(
                                out=pt[:], in_=ut[:], func=EXP,
                                bias=umin[:], scale=-1.0)
                            half = ptbA if it < 4 else ptbB
                            hslot = slice((it % 4) * P, (it % 4) * P + P)
                            nc.sync.dma_start_transpose(half[:, :, hslot], pt[:])

                    def attn_pv(h):
                        ptb_halves = ptbs.pop(h)
                        csl = slice((h // 2) * P + (h % 2) * 64,
                                    (h // 2) * P + (h % 2) * 64 + 64)
                        obase = (h % 2) * 64
                        for nh in range(2):
                            hsl = slice(nh * 512, (nh + 1) * 512)
                            half = ptb_halves[nh]
                            pso = ps_o.tile([64, 512], F32, tag="pso")
                            for jt in range(NT):
                                nc.tensor.matmul(
                                    pso[:], vmat[:, jt, csl], half[:, jt, :],
                                    start=(jt == 0), stop=(jt == NT - 1))
                            og = ostg.tile([64, 512], BF16, tag="og")
                            nc.scalar.copy(og[:], pso[:])
                            nc.sync.dma_start(
                                otm_d[obase:obase + 64, h // 2, hsl], og[:])

                    # attention lags projection by one head-pair so the
                    # proj-subs (DVE) and copies (Act) for the next heads are
                    # issued ahead of the current heads' stt/reduce/exp
                    # backlog on those same in-order engines.
                    proj_et(0)
                    proj_et(8)
                    for i in range(1, 8):
                        proj_et(i)
                        proj_et(8 + i)
                        attn_front(2 * i - 2)
                        if i > 1:
                            attn_pv(2 * i - 3)
                        attn_front(2 * i - 1)
                        attn_pv(2 * i - 2)
                    attn_front(14)
                    attn_pv(13)
                    attn_front(15)
                    attn_pv(14)
                    attn_pv(15)

            # ---------------- phase 4: output projection ----------------
            with tc.tile_pool(name="late", bufs=1) as late, \
                 tc.tile_pool(name="ypool", bufs=2) as ypool:
                wpt = late.tile([P, NT, D], BF16, tag="wp")
                bp = late.tile([1, D], BF16, tag="bp")
                bpb = late.tile([P, D], BF16, tag="bpb")
                otm = late.tile([P, NT, S], BF16, tag="otm")
                nc.scalar.dma_start(
                    wpt[:], wp_d.ap().rearrange("(t p) d -> p t d", p=P))
                nc.scalar.dma_start(bp[:], bp_d[:])
                # per-ot reload: each c-tile only waits for its two heads' PV
                # (same sync queue as the og writes: FIFO-safe)
                for ot in range(NT):
                    nc.sync.dma_start(otm[:, ot, :], otm_d[:, ot, :])
                for half in range(2):
                    hsl = slice(half * 512, (half + 1) * 512)
                    bps = psA.tile([P, 512], F32, tag="ps")
                    nc.tensor.matmul(bps[:], ones_row[:], bp[:, hsl],
                                     start=True, stop=True)
                    nc.scalar.copy(bpb[:, hsl], bps[:])
                for st in range(NT):
                    ssl = slice(st * P, (st + 1) * P)
                    yt = ypool.tile([P, D], F32, tag="yt")
                    for nh in range(2):
                        hsl = slice(nh * 512, (nh + 1) * 512)
                        ps = psA.tile([P, 512], F32, tag="ps")
                        for ot in range(NT):
                            nc.tensor.matmul(
                                ps[:], otm[:, ot, ssl], wpt[:, ot, hsl],
                                start=(ot == 0), stop=(ot == NT - 1))
                        nc.vector.scalar_tensor_tensor(
                            out=yt[:, hsl], in0=ps[:], scalar=0.0,
                            in1=bpb[:, hsl], op0=ADD, op1=ADD)
                    nc.sync.dma_start(y_d[st * P:(st + 1) * P, :], yt[:])

    nc.compile()
    return nc


def _prep_inputs(x, mask, W_attn, b_attn, W_proj, b_proj):
    x = np.asarray(x, np.float32)
    mask = np.asarray(mask, np.float32)
    W_attn = np.asarray(W_attn, np.float32)
    b_attn = np.asarray(b_attn, np.float32).reshape(-1)
    W_proj = np.asarray(W_proj, np.float32)
    b_proj = np.asarray(b_proj, np.float32).reshape(-1)

    wqk = W_attn[:, : 2 * D]
    wqkh = _bf16(wqk)
    wqkl = _bf16(wqk - wqkh.astype(np.float32))
    wv = _bf16(W_attn[:, 2 * D:])
    wp = _bf16(W_proj)

    bqk = b_attn[: 2 * D].reshape(16, 128)  # [et, p]
    bqk_lo = np.ascontiguousarray(bqk[:, 0:64].T, np.float32)  # [64, 16]
    bqk_hi = np.ascontiguousarray(bqk[:, 64:128].T, np.float32)
    bv = _bf16(b_attn[2 * D:].reshape(1, D))
    bp = _bf16(b_proj.reshape(1, D))

    shared = dict(wqkh=wqkh, wqkl=wqkl, wv=wv, wp=wp,
                  bqk_lo=bqk_lo, bqk_hi=bqk_hi, bv=bv, bp=bp)
    in_maps = []
    for b in range(B):
        xT = np.ascontiguousarray(x[b].T)
        xh = _bf16(xT)
        xli = _bf16(xT - xh.astype(np.float32))
        in_maps.append(dict(
            xh=xh, xl=xli,
            m=np.ascontiguousarray(mask[b, 0].astype(ml_dtypes.float8_e4m3)),
            **shared))
    return in_maps


def kernel(x, mask, W_attn, b_attn, W_proj, b_proj, _trace=False, _trace_kwargs=None):
    if "nc" not in _CACHE:
        _CACHE["nc"] = _build()
    nc = _CACHE["nc"]
    in_maps = _prep_inputs(x, mask, W_attn, b_attn, W_proj, b_proj)
    kw = {}
    if _trace:
        kw = dict(trace=True, **(_trace_kwargs or {}))
    res = run_bass_kernel_spmd(nc, in_maps, core_ids=list(range(B)), **kw)
    out = np.stack([res.results[b]["y"] for b in range(B)], axis=0)
    if _trace:
        _CACHE["last_results"] = res
    return out
